# revision 17
# baseline (speedup 1.0000x reference)
"""Trainium2 kernel for Conv2d_cd (central-difference conv, 3x3, theta=0.7).

Reference math:
    s = sum of 9 shifted views of reflect-padded x  (= 3x3 box filter, reflect pad)
    out = conv3x3_zeropad(s, W) - theta * conv1x1(s, W.sum((2,3)))
        = conv3x3_zeropad(s, W')     with W'[:,:,1,1] -= theta * W.sum((2,3))

Strategy (per NeuronCore, 8 cores data-parallel over batch 16 -> 2 images/core):
  - images stacked on SBUF partition halves: partitions 0:64 = img0 ch, 64:128 = img1 ch
  - H strips; per strip: cast x->bf16 on ScalarE, V-box (2 bf16 2x adds on DVE).
    F strips: H-box pair-add split GpSimd/DVE, center-add (2x) on DVE deferred one
    strip (so the GpSimd part never stalls the DVE queue), then 9 conv taps.
    T strips (ends): H-box folded into 15 taps.  Strips share their 2 boundary s
    rows via a tiny copy instead of recomputation.
  - taps as K=64/M=64 matmuls packed 4-concurrent in PE quadrants into one
    [128,2048] PSUM tile; chunk->row permutation [0,2,1,3] makes each store
    destination 8 contiguous DRAM rows.  Evacuation: single ScalarE copy to fp16
    (output stored as fp16, widened to fp32 on host; rel-err budget 2e-2).
"""

import os

import numpy as np
import ml_dtypes

import concourse.bass as bass
import concourse.bacc as bacc
import concourse.mybir as mybir
from concourse.tile import TileContext
from concourse.bass_utils import run_bass_kernel_spmd

THETA = 0.7
N_CORES = 8
B, C, H, W = 16, 64, 128, 128
BPC = B // N_CORES          # images per core = 2
WP = W + 4                  # padded width of s tiles (132)
F32 = mybir.dt.float32
BF16 = mybir.dt.bfloat16
F16 = mybir.dt.float16


def _parse_plan():
    # kinds: T = folded (15-tap), F = unfolded (9-tap + H on DVE),
    # I = independent folded bottom strip (rows [128-R, 128), computes its
    # own boundary rows; lets the bottom taps run early instead of last)
    txt = os.environ.get("K_PLAN", "F8,J8,F16,F16,F16,F16,F16,F16,F8,F8")
    plan = []
    for item in txt.split(","):
        item = item.strip()
        plan.append((int(item[1:]), item[0] in "TI", item[0] in "IJ"))
    assert sum(r for r, _, _ in plan) == H, plan
    return plan


PLAN = _parse_plan()
OUT16 = os.environ.get("K_OUT16", "1") == "1"   # store output as fp16
H1_POOL_ROWS = int(os.environ.get("K_H1_POOL_ROWS", "0"))  # of S=18 on GpSimd
STORE_Q = os.environ.get("K_STORE_Q", "ssa")  # a=scalar g=gpsimd s=sync v=vector
DMACAST = os.environ.get("K_DMACAST", "1") == "1"  # f32->bf16 cast in SWDGE DMA
XBUFS = int(os.environ.get("K_XBUFS", str(len(PLAN) if DMACAST else 4)))
SBUFS = int(os.environ.get("K_SBUFS", "4"))
OBUFS = int(os.environ.get("K_OBUFS", "3"))
PBUFS = int(os.environ.get("K_PBUFS", "4"))    # psum rotation slots (2 banks ea)
CHUNK = int(os.environ.get("K_CHUNK", "8"))    # tap/evac chunk rows
ODT = F16 if OUT16 else F32
EDGE_SC = os.environ.get("K_EDGE_SC", "1" if DMACAST else "0") == "1"
ZOV = os.environ.get("K_ZOV", "0") == "1"
HEAD_SC = os.environ.get("K_HEAD_SC", "1" if DMACAST else "0") == "1"
L0Q = os.environ.get("K_L0Q", "s")  # first-strip load ring: s=sync a=scalar


def _host_weights(Wnp: np.ndarray):
    """W' and packed tap weights: wt[p, t*C + co], taps 0..8 = w9 (ky*3+kx),
    taps 9..23 = w15 (ky*5+tx); both partition halves identical."""
    Wp = Wnp.astype(np.float64).copy()
    Wp[:, :, 1, 1] -= THETA * Wnp.astype(np.float64).sum(axis=(2, 3))
    w9 = Wp.transpose(1, 2, 3, 0).reshape(C, 9, C)
    w15 = np.zeros((C, 3, 5, C), np.float64)
    for ky in range(3):
        for tx in range(5):
            for kx in range(max(0, tx - 2), min(2, tx) + 1):
                w15[:, ky, tx, :] += Wp[:, :, ky, kx].T  # [ci, co]
    wt = np.concatenate([w9.reshape(C, 9 * C), w15.reshape(C, 15 * C)], axis=1)
    wt = np.concatenate([wt, wt], axis=0)
    return np.ascontiguousarray(wt.astype(ml_dtypes.bfloat16))


def build():
    nc = bacc.Bacc("TRN2", target_bir_lowering=False, debug=False,
                   num_devices=N_CORES)
    x_d = nc.declare_dram_parameter("x", [BPC, C, H, W], F32, isOutput=False)
    wt_d = nc.declare_dram_parameter("wt", [128, 24 * C], BF16, isOutput=False)
    out_d = nc.declare_dram_parameter("out", [BPC, C, H, W], ODT, isOutput=True)

    x_pc = x_d.rearrange("i c h w -> (i c) h w")
    # store views: 8 (or 4) contiguous rows per (img, ch) descriptor
    out8 = out_d.rearrange("i c (g8 eight) w -> i c g8 (eight w)", eight=8)
    out4 = out_d.rearrange("i c (g4 four) w -> i c g4 (four w)", four=4)
    out8pc = out_d.rearrange("i c (g8 eight) w -> (i c) g8 (eight w)", eight=8)
    out4pc = out_d.rearrange("i c (g4 four) w -> (i c) g4 (four w)", four=4)

    pam = os.environ.get("K_POOL_MODE", "queue")
    with TileContext(nc, pool_alloc_mode=pam) as tc:
        with (
            tc.tile_pool(name="wpool", bufs=1) as wpool,
            tc.tile_pool(name="warmpool", bufs=1) as warmpool,
            tc.tile_pool(name="xpool", bufs=XBUFS) as xpool,
            tc.tile_pool(name="bpool", bufs=XBUFS) as bpool,
            tc.tile_pool(name="upool", bufs=2) as upool,
            tc.tile_pool(name="vpool", bufs=2) as vpool,
            tc.tile_pool(name="tpool", bufs=2) as tpool,
            tc.tile_pool(name="spool", bufs=SBUFS) as spool,
            tc.tile_pool(name="opool", bufs=OBUFS) as opool,
            tc.tile_pool(name="psum", bufs=PBUFS, space="PSUM") as ppool,
        ):
            wt_sb = wpool.tile([128, 24 * C], BF16)
            nc.scalar.dma_start(out=wt_sb[:], in_=wt_d[:])

            # PE p-state warmup: dep-free matmuls keep the tensor engine
            # busy through the load phase so the first real taps run at
            # full clock (ramp needs ~3us of sustained execution)
            NWARM = int(os.environ.get("K_WARM", "0"))
            warm_pt = None
            if NWARM:
                wrm = warmpool.tile([128, 512], BF16)
                nc.vector.memset(wrm[:], 0.0)
                NP0 = PLAN[0][0] * W // 1024
                warm_pt = ppool.tile([128, 1024 * NP0], F32, tag="ps",
                                     name="warm_ps")
                for _ in range(NWARM):
                    nc.tensor.matmul(warm_pt[0:64, 0:512], wrm[0:64, 0:64],
                                     wrm[0:64, :], start=True, stop=True,
                                     skip_group_check=True)

            qmap = {"a": nc.scalar, "g": nc.gpsimd, "s": nc.sync,
                    "v": nc.vector}

            def emit_taps(s3, folded, NP, r0, si, pt=None, row_off=0):
                # psum col block j holds output rows [4j, 4j+4) (row-major).
                # Per (pair, tap): 4 quadrant matmuls on blocks {p, p+NP}:
                # img i -> half i on block p, half 1-i on block p+NP, so ot
                # halves end up [img rows 0..R/2 | img rows R/2..R] contiguous.
                if pt is None:
                    pt = ppool.tile([128, 1024 * NP], F32, tag="ps",
                                    name=f"ps{si}")
                ntap = 15 if folded else 9
                tap0 = 9 if folded else 0
                nkx = 5 if folded else 3
                cofs = 0 if folded else 1
                for p in range(NP):
                    for t in range(ntap):
                        ky, kx = t // nkx, t % nkx
                        tw = tap0 + t
                        for (i, j, pbase) in ((0, p, 0), (1, p, 64),
                                              (0, p + NP, 64), (1, p + NP, 0)):
                            rhs = s3[64 * i:64 * i + 64,
                                     row_off + 4 * j + ky:
                                     row_off + 4 * j + ky + 4,
                                     kx + cofs:kx + cofs + 128]
                            nc.tensor.matmul(
                                pt[pbase:pbase + 64, 512 * j:512 * j + 512],
                                wt_sb[64 * i:64 * i + 64, tw * C:(tw + 1) * C],
                                rhs,
                                start=(t == 0), stop=(t == ntap - 1),
                                skip_group_check=True,
                            )
                return pt

            def emit_evac_store(pt, R, NP, r0, split=False):
                ncol = R * W
                hc = 512 * NP   # half of the strip's rows
                ot = opool.tile([128, ncol], ODT, tag="ot", name=f"ot{r0}")
                if NP == 2:
                    dv, dvpc, g = out8, out8pc, r0 // 8
                else:
                    dv, dvpc, g = out4, out4pc, r0 // 4
                stores = [
                    (dvpc[:, g + 0, :], ot[:, 0:hc]),
                    (dv[0, :, g + 1, :], ot[64:128, hc:2 * hc]),
                    (dv[1, :, g + 1, :], ot[0:64, hc:2 * hc]),
                ]
                if split:
                    # tail strips: halves evacuated on ScalarE/DVE in parallel,
                    # stores chase each half; all on HWDGE queues so no
                    # GpSimd SWDGE drain delays the kernel end
                    nc.scalar.copy(out=ot[:, 0:hc], in_=pt[:, 0:hc])
                    nc.sync.dma_start(out=stores[0][0], in_=stores[0][1])
                    nc.vector.tensor_copy(out=ot[:, hc:ncol],
                                          in_=pt[:, hc:ncol])
                    nc.sync.dma_start(out=stores[1][0], in_=stores[1][1])
                    nc.scalar.dma_start(out=stores[2][0], in_=stores[2][1])
                else:
                    nc.scalar.copy(out=ot[:], in_=pt[:, 0:ncol])
                    for (dst, srcp), qc in zip(stores, STORE_Q):
                        qmap[qc].dma_start(out=dst, in_=srcp)

            pend_h2 = None    # (sv3, th3, s3, NP, r0, si) F strip awaiting H2+taps
            evac_fifo = []    # [(pt, R, NP, r0)]
            prev_sv = None    # (tile3, S, is_T): sv-valued buffer of prev strip
            prev_xb = None    # (xb3, nxr): previous strip's cast x tile

            has_indep = any(ind for _, _, ind in PLAN)

            # ---- strip geometry (plan order) ----
            geom = []
            r0n = 0
            for si, (R, folded, indep) in enumerate(PLAN):
                if indep:
                    r0 = H - R
                else:
                    r0 = r0n
                    r0n += R
                if indep:
                    x_lo = r0 - 2
                elif si == 0:
                    x_lo = r0
                else:
                    # ZOV=1: zero-overlap loads (min DMA, +2 small DVE ops);
                    # ZOV=0: 2-row halo re-load (min DVE ops, +1 MB DMA)
                    x_lo = r0 + 2 if ZOV else r0
                x_hi = min(H, r0 + R + 2)
                geom.append((r0, x_lo, x_hi))

            # ---- loads: SWDGE cast-DMA (f32 DRAM -> bf16 SBUF), all
            # pre-emitted so descriptor generation runs once up front and
            # every strip's data streams in as early as HBM bandwidth allows
            pre_xb = [None] * len(PLAN)
            if DMACAST:
                for si in range(len(PLAN)):
                    r0, x_lo, x_hi = geom[si]
                    nxr = x_hi - x_lo
                    xb = bpool.tile([128, nxr * W], BF16, tag="xb")
                    # all loads on the single Pool SWDGE ring: FIFO per ring,
                    # so strip 0 streams first at full bandwidth
                    nc.gpsimd.dma_start(out=xb[:], in_=x_pc[:, x_lo:x_hi, :])
                    pre_xb[si] = xb

            for si, (R, folded, indep) in enumerate(PLAN):
                first = si == 0
                r0, x_lo, x_hi = geom[si]
                # geometric bottom strip: handles row-127 reflect + zero row
                last = indep or (not has_indep and si == len(PLAN) - 1)
                r1 = r0 + R
                S = R + 2                     # s rows: positions [r0-1, r1+1)
                NP = R * W // 1024

                nxr = x_hi - x_lo
                if DMACAST:
                    xb = pre_xb[si]
                else:
                    xt = xpool.tile([128, nxr * W], F32, tag="xt")
                    xb = bpool.tile([128, nxr * W], BF16, tag="xb")
                    if first:
                        # split load + DVE cast (no ACT_TABLE_LOAD dependency)
                        # so the first strip's chain starts as early as can be
                        mid = nxr // 2
                        nc.sync.dma_start(out=xt[:, 0:mid * W],
                                          in_=x_pc[:, x_lo:x_lo + mid, :])
                        nc.sync.dma_start(out=xt[:, mid * W:],
                                          in_=x_pc[:, x_lo + mid:x_hi, :])
                        nc.vector.tensor_copy(out=xb[:, 0:mid * W],
                                              in_=xt[:, 0:mid * W])
                        nc.vector.tensor_copy(out=xb[:, mid * W:],
                                              in_=xt[:, mid * W:])
                    else:
                        nc.sync.dma_start(out=xt[:], in_=x_pc[:, x_lo:x_hi, :])
                        nc.scalar.copy(out=xb[:], in_=xt[:])
                xb3 = xb.rearrange("p (r w) -> p r w", w=W)

                # ---- V-box: sv[j] = x[j-1]+x[j]+x[j+1], reflect at 0/127 ----
                jlo = 0 if first else (r0 - 1 if indep else r0 + 1)
                jhi = H if last else r1 + 1
                nu = jhi - jlo
                jm_lo = max(jlo, x_lo + 1)
                jm_hi = min(jhi, x_hi - 1)
                ut = upool.tile([128, nu * W], BF16, tag="ut")
                u3 = ut.rearrange("p (r w) -> p r w", w=W)
                nc.vector.tensor_add(
                    out=u3[:, jm_lo - jlo:jm_hi - jlo, :],
                    in0=xb3[:, jm_lo - 1 - x_lo:jm_hi - 1 - x_lo, :],
                    in1=xb3[:, jm_lo + 1 - x_lo:jm_hi + 1 - x_lo, :])
                xtile = jm_lo > jlo and not first and not indep
                if xtile:
                    # boundary rows j in [jlo, jm_lo): x[j-1] from prev tile
                    pxb3, pnxr = prev_xb
                    nb = jm_lo - jlo   # == 2
                    nc.vector.tensor_add(
                        out=u3[:, 0:nb, :],
                        in0=pxb3[:, pnxr - nb:pnxr, :],
                        in1=xb3[:, jlo + 1 - x_lo:jm_lo + 1 - x_lo, :])
                if first:
                    nc.vector.tensor_scalar_mul(
                        out=u3[:, 0:1, :], in0=xb3[:, 1:2, :], scalar1=2.0)
                if last:
                    nc.vector.tensor_scalar_mul(
                        out=u3[:, nu - 1:nu, :],
                        in0=xb3[:, 126 - x_lo:127 - x_lo, :], scalar1=2.0)
                if not indep:
                    prev_xb = (xb3, nxr)

                st = spool.tile([128, S * WP], BF16, tag="st")
                s3 = st.rearrange("p (r c) -> p r c", c=WP)
                brow = jlo - (r0 - 1)
                if folded:
                    sv3 = None
                    v2out = lambda a, b: s3[:, brow + a:brow + b, 2:130]
                else:
                    svt = vpool.tile([128, S * W], BF16, tag="svt")
                    sv3 = svt.rearrange("p (r w) -> p r w", w=W)
                    v2out = lambda a, b: sv3[:, brow + a:brow + b, :]
                if xtile:
                    # sv[jlo]: center tap x[jlo] is prev tile's last row
                    nc.vector.tensor_add(
                        out=v2out(0, 1), in0=u3[:, 0:1, :],
                        in1=pxb3[:, pnxr - 1:pnxr, :])
                    nc.vector.tensor_add(
                        out=v2out(1, nu), in0=u3[:, 1:nu, :],
                        in1=xb3[:, jlo + 1 - x_lo:jhi - x_lo, :])
                else:
                    nc.vector.tensor_add(
                        out=v2out(0, nu), in0=u3[:, :, :],
                        in1=xb3[:, jlo - x_lo:jhi - x_lo, :])

                # ---- head rows [r0-1, r0+1): copy prev strip's last 2 sv rows
                if not first and not indep:
                    psv3, pS, p_folded = prev_sv
                    src = (psv3[:, pS - 2:pS, 2:130] if p_folded
                           else psv3[:, pS - 2:pS, :])
                    dst = (s3[:, 0:2, 2:130] if folded else sv3[:, 0:2, :])
                    if HEAD_SC:
                        nc.scalar.copy(out=dst, in_=src)
                    else:
                        nc.vector.tensor_copy(out=dst, in_=src)
                if not indep:
                    prev_sv = ((s3, S, True) if folded else (sv3, S, False))

                # ---- conv zero-pad rows at image top/bottom (before the
                # side-col fixups, which read all rows) ----
                if first:
                    nc.vector.memset(s3[:, 0:1, :], 0.0)
                if last:
                    nc.vector.memset(s3[:, S - 1:S, :], 0.0)

                # valid sv rows for th/H2: row 0 is the conv zero row on the
                # first strip, row S-1 on the geometric bottom strip
                h2lo = brow if first else 0
                h2hi = S - 1 if last else S
                if not folded:
                    # ---- H-box pair-add: th[w] = sv[w-1]+sv[w+1] ----
                    tht = tpool.tile([128, S * W], BF16, tag="tht")
                    th3 = tht.rearrange("p (r w) -> p r w", w=W)
                    hp = max(h2lo, min(h2hi, h2lo + H1_POOL_ROWS))
                    if hp > h2lo:
                        nc.gpsimd.tensor_add(out=th3[:, h2lo:hp, 1:127],
                                             in0=sv3[:, h2lo:hp, 0:126],
                                             in1=sv3[:, h2lo:hp, 2:128])
                    if hp < h2hi:
                        nc.vector.tensor_add(out=th3[:, hp:h2hi, 1:127],
                                             in0=sv3[:, hp:h2hi, 0:126],
                                             in1=sv3[:, hp:h2hi, 2:128])
                    if EDGE_SC:
                        nc.scalar.mul(th3[:, h2lo:h2hi, 0:1],
                                      sv3[:, h2lo:h2hi, 1:2], 2.0)
                        nc.scalar.mul(th3[:, h2lo:h2hi, 127:128],
                                      sv3[:, h2lo:h2hi, 126:127], 2.0)
                    else:
                        nc.vector.tensor_scalar_mul(
                            out=th3[:, h2lo:h2hi, 0:1],
                            in0=sv3[:, h2lo:h2hi, 1:2], scalar1=2.0)
                        nc.vector.tensor_scalar_mul(
                            out=th3[:, h2lo:h2hi, 127:128],
                            in0=sv3[:, h2lo:h2hi, 126:127], scalar1=2.0)
                    nc.vector.memset(s3[:, :, 0:2], 0.0)
                    nc.vector.memset(s3[:, :, 130:132], 0.0)
                else:
                    # ---- folded side cols: col c reads sv[c-2] ----
                    nc.vector.tensor_copy(out=s3[:, :, 1:2], in_=s3[:, :, 3:4])
                    nc.vector.tensor_copy(out=s3[:, :, 130:131],
                                          in_=s3[:, :, 128:129])
                    nc.vector.scalar_tensor_tensor(
                        out=s3[:, :, 0:1], in0=s3[:, :, 2:3], scalar=-1.0,
                        in1=s3[:, :, 3:4], op0=mybir.AluOpType.mult,
                        op1=mybir.AluOpType.subtract)
                    nc.vector.scalar_tensor_tensor(
                        out=s3[:, :, 131:132], in0=s3[:, :, 129:130],
                        scalar=-1.0, in1=s3[:, :, 128:129],
                        op0=mybir.AluOpType.mult,
                        op1=mybir.AluOpType.subtract)

                if folded:
                    pt = emit_taps(s3, True, NP, r0, si,
                                   pt=warm_pt if first else None)
                    evac_fifo.append((pt, R, NP, r0))

                def emit_h2_taps(dsv3, dth3, ds3, dR, dr0, dsi, dlo, dhi):
                    nc.vector.tensor_add(out=ds3[:, dlo:dhi, 2:130],
                                         in0=dth3[:, dlo:dhi, 0:128],
                                         in1=dsv3[:, dlo:dhi, 0:128])
                    # taps/evac/stores in CHUNK-row blocks (psum slots)
                    for h, off in enumerate(range(0, dR, CHUNK)):
                        rows = min(CHUNK, dR - off)
                        cnp = rows * W // 1024
                        pt = emit_taps(ds3, False, cnp, dr0 + off,
                                       10 * dsi + h, row_off=off)
                        evac_fifo.append((pt, rows, cnp, dr0 + off))
                        while len(evac_fifo) > 1:
                            emit_evac_store(*evac_fifo.pop(0))

                # ---- deferred H2 + taps of the previous F strip (emitted
                # after a folded strip's taps so those hide under the H2) ----
                if pend_h2 is not None:
                    emit_h2_taps(*pend_h2)
                    pend_h2 = None

                if folded:
                    pass
                elif H1_POOL_ROWS == 0 and not (si + 1 < len(PLAN)
                                                and PLAN[si + 1][1]):
                    # H1 all on DVE: no cross-engine stall risk, emit inline
                    emit_h2_taps(sv3, th3, s3, R, r0, si, h2lo, h2hi)
                else:
                    # last F strip (or H1_POOL_ROWS mode): defer H2+taps
                    pend_h2 = (sv3, th3, s3, R, r0, si, h2lo, h2hi)

                while len(evac_fifo) > 1:
                    emit_evac_store(*evac_fifo.pop(0))

            if pend_h2 is not None:
                emit_h2_taps(*pend_h2)
            while evac_fifo:
                emit_evac_store(*evac_fifo.pop(0), split=True)

    nc.compile()
    return nc


_CACHE = {}


def _get_nc():
    if "nc" not in _CACHE:
        _CACHE["nc"] = build()
    return _CACHE["nc"]


def kernel(x: np.ndarray, W: np.ndarray, trace: bool = False):
    x = np.asarray(x, dtype=np.float32)
    wt = _host_weights(np.asarray(W, dtype=np.float32))
    nc = _get_nc()
    core_ids = list(range(N_CORES))
    in_maps = [
        {"x": np.ascontiguousarray(x[BPC * i:BPC * (i + 1)]), "wt": wt}
        for i in core_ids
    ]
    res = run_bass_kernel_spmd(nc, in_maps, core_ids, trace=trace)
    out = np.concatenate(
        [res.results[i]["out"].astype(np.float32) for i in core_ids], axis=0)
    if trace:
        kernel.last_exec_time_ns = res.exec_time_ns
        kernel.last_res = res
    return out


kernel.last_exec_time_ns = None



# revision 20
# speedup vs baseline: 1.0386x; 1.0386x over previous
"""Trainium2 kernel for Conv2d_cd (central-difference conv, 3x3, theta=0.7).

Reference math:
    s = sum of 9 shifted views of reflect-padded x  (= 3x3 box filter, reflect pad)
    out = conv3x3_zeropad(s, W) - theta * conv1x1(s, W.sum((2,3)))
        = conv3x3_zeropad(s, W')     with W'[:,:,1,1] -= theta * W.sum((2,3))

Strategy (per NeuronCore, 8 cores data-parallel over batch 16 -> 2 images/core):
  - images stacked on SBUF partition halves: partitions 0:64 = img0 ch, 64:128 = img1 ch
  - H strips; per strip: cast x->bf16 on ScalarE, V-box (2 bf16 2x adds on DVE).
    F strips: H-box pair-add split GpSimd/DVE, center-add (2x) on DVE deferred one
    strip (so the GpSimd part never stalls the DVE queue), then 9 conv taps.
    T strips (ends): H-box folded into 15 taps.  Strips share their 2 boundary s
    rows via a tiny copy instead of recomputation.
  - taps as K=64/M=64 matmuls packed 4-concurrent in PE quadrants into one
    [128,2048] PSUM tile; chunk->row permutation [0,2,1,3] makes each store
    destination 8 contiguous DRAM rows.  Evacuation: single ScalarE copy to fp16
    (output stored as fp16, widened to fp32 on host; rel-err budget 2e-2).
"""

import os

import numpy as np
import ml_dtypes

import concourse.bass as bass
import concourse.bacc as bacc
import concourse.mybir as mybir
from concourse.tile import TileContext
from concourse.bass_utils import run_bass_kernel_spmd

THETA = 0.7
N_CORES = 8
B, C, H, W = 16, 64, 128, 128
BPC = B // N_CORES          # images per core = 2
WP = W + 4                  # padded width of s tiles (132)
F32 = mybir.dt.float32
BF16 = mybir.dt.bfloat16
F16 = mybir.dt.float16


def _parse_plan():
    # kinds: T = folded (15-tap), F = unfolded (9-tap + H on DVE),
    # I = independent folded bottom strip (rows [128-R, 128), computes its
    # own boundary rows; lets the bottom taps run early instead of last)
    txt = os.environ.get("K_PLAN", "F8,J8,F16,F16,F16,F16,F16,F16,F8,F8")
    plan = []
    for item in txt.split(","):
        item = item.strip()
        plan.append((int(item[1:]), item[0] in "TI", item[0] in "IJ"))
    assert sum(r for r, _, _ in plan) == H, plan
    return plan


PLAN = _parse_plan()
OUT16 = os.environ.get("K_OUT16", "1") == "1"   # store output as fp16
H1_POOL_ROWS = int(os.environ.get("K_H1_POOL_ROWS", "0"))  # of S=18 on GpSimd
STORE_Q = os.environ.get("K_STORE_Q", "ssa")  # a=scalar g=gpsimd s=sync v=vector
DMACAST = os.environ.get("K_DMACAST", "1") == "1"  # f32->bf16 cast in SWDGE DMA
XBUFS = int(os.environ.get("K_XBUFS", str(len(PLAN) if DMACAST else 4)))
SBUFS = int(os.environ.get("K_SBUFS", "4"))
OBUFS = int(os.environ.get("K_OBUFS", "3"))
PBUFS = int(os.environ.get("K_PBUFS", "4"))    # psum rotation slots (2 banks ea)
CHUNK = int(os.environ.get("K_CHUNK", "8"))    # tap/evac chunk rows
ODT = F16 if OUT16 else F32
EDGE_SC = os.environ.get("K_EDGE_SC", "1" if DMACAST else "0") == "1"
ZOV = os.environ.get("K_ZOV", "0") == "1"
HEAD_SC = os.environ.get("K_HEAD_SC", "1" if DMACAST else "0") == "1"
L0Q = os.environ.get("K_L0Q", "s")  # first-strip load ring: s=sync a=scalar


def _host_weights(Wnp: np.ndarray):
    """W' and packed tap weights: wt[p, t*C + co], taps 0..8 = w9 (ky*3+kx),
    taps 9..23 = w15 (ky*5+tx); both partition halves identical."""
    Wp = Wnp.astype(np.float64).copy()
    Wp[:, :, 1, 1] -= THETA * Wnp.astype(np.float64).sum(axis=(2, 3))
    w9 = Wp.transpose(1, 2, 3, 0).reshape(C, 9, C)
    w15 = np.zeros((C, 3, 5, C), np.float64)
    for ky in range(3):
        for tx in range(5):
            for kx in range(max(0, tx - 2), min(2, tx) + 1):
                w15[:, ky, tx, :] += Wp[:, :, ky, kx].T  # [ci, co]
    wt = np.concatenate([w9.reshape(C, 9 * C), w15.reshape(C, 15 * C)], axis=1)
    wt = np.concatenate([wt, wt], axis=0)
    return np.ascontiguousarray(wt.astype(ml_dtypes.bfloat16))


def build():
    nc = bacc.Bacc("TRN2", target_bir_lowering=False, debug=False,
                   num_devices=N_CORES)
    x_d = nc.declare_dram_parameter("x", [BPC, C, H, W], F32, isOutput=False)
    wt_d = nc.declare_dram_parameter("wt", [128, 24 * C], BF16, isOutput=False)
    out_d = nc.declare_dram_parameter("out", [BPC, C, H, W], ODT, isOutput=True)

    x_pc = x_d.rearrange("i c h w -> (i c) h w")
    # store views: 8 (or 4) contiguous rows per (img, ch) descriptor
    out8 = out_d.rearrange("i c (g8 eight) w -> i c g8 (eight w)", eight=8)
    out4 = out_d.rearrange("i c (g4 four) w -> i c g4 (four w)", four=4)
    out8pc = out_d.rearrange("i c (g8 eight) w -> (i c) g8 (eight w)", eight=8)
    out4pc = out_d.rearrange("i c (g4 four) w -> (i c) g4 (four w)", four=4)

    pam = os.environ.get("K_POOL_MODE", "queue")
    with TileContext(nc, pool_alloc_mode=pam) as tc:
        with (
            tc.tile_pool(name="wpool", bufs=1) as wpool,
            tc.tile_pool(name="warmpool", bufs=1) as warmpool,
            tc.tile_pool(name="xpool", bufs=XBUFS) as xpool,
            tc.tile_pool(name="bpool", bufs=XBUFS) as bpool,
            tc.tile_pool(name="upool", bufs=2) as upool,
            tc.tile_pool(name="vpool", bufs=2) as vpool,
            tc.tile_pool(name="tpool", bufs=2) as tpool,
            tc.tile_pool(name="spool", bufs=SBUFS) as spool,
            tc.tile_pool(name="opool", bufs=OBUFS) as opool,
            tc.tile_pool(name="psum", bufs=PBUFS, space="PSUM") as ppool,
        ):
            wt_sb = wpool.tile([128, 24 * C], BF16)
            (nc.sync if L0Q == "a" else nc.scalar).dma_start(
                out=wt_sb[:], in_=wt_d[:])

            # PE p-state warmup: dep-free matmuls keep the tensor engine
            # busy through the load phase so the first real taps run at
            # full clock (ramp needs ~3us of sustained execution)
            NWARM = int(os.environ.get("K_WARM", "0"))
            warm_pt = None
            if NWARM:
                wrm = warmpool.tile([128, 512], BF16)
                nc.vector.memset(wrm[:], 0.0)
                NP0 = PLAN[0][0] * W // 1024
                warm_pt = ppool.tile([128, 1024 * NP0], F32, tag="ps",
                                     name="warm_ps")
                for _ in range(NWARM):
                    nc.tensor.matmul(warm_pt[0:64, 0:512], wrm[0:64, 0:64],
                                     wrm[0:64, :], start=True, stop=True,
                                     skip_group_check=True)

            qmap = {"a": nc.scalar, "g": nc.gpsimd, "s": nc.sync,
                    "v": nc.vector}

            def emit_taps(s3, folded, NP, r0, si, pt=None, row_off=0):
                # psum col block j holds output rows [4j, 4j+4) (row-major).
                # Per (pair, tap): 4 quadrant matmuls on blocks {p, p+NP}:
                # img i -> half i on block p, half 1-i on block p+NP, so ot
                # halves end up [img rows 0..R/2 | img rows R/2..R] contiguous.
                if pt is None:
                    pt = ppool.tile([128, 1024 * NP], F32, tag="ps",
                                    name=f"ps{si}")
                ntap = 15 if folded else 9
                tap0 = 9 if folded else 0
                nkx = 5 if folded else 3
                cofs = 0 if folded else 1
                for p in range(NP):
                    for t in range(ntap):
                        ky, kx = t // nkx, t % nkx
                        tw = tap0 + t
                        for (i, j, pbase) in ((0, p, 0), (1, p, 64),
                                              (0, p + NP, 64), (1, p + NP, 0)):
                            rhs = s3[64 * i:64 * i + 64,
                                     row_off + 4 * j + ky:
                                     row_off + 4 * j + ky + 4,
                                     kx + cofs:kx + cofs + 128]
                            nc.tensor.matmul(
                                pt[pbase:pbase + 64, 512 * j:512 * j + 512],
                                wt_sb[64 * i:64 * i + 64, tw * C:(tw + 1) * C],
                                rhs,
                                start=(t == 0), stop=(t == ntap - 1),
                                skip_group_check=True,
                            )
                return pt

            def emit_evac_store(pt, R, NP, r0, split=False):
                ncol = R * W
                hc = 512 * NP   # half of the strip's rows
                ot = opool.tile([128, ncol], ODT, tag="ot", name=f"ot{r0}")
                if NP == 2:
                    dv, dvpc, g = out8, out8pc, r0 // 8
                else:
                    dv, dvpc, g = out4, out4pc, r0 // 4
                stores = [
                    (dvpc[:, g + 0, :], ot[:, 0:hc]),
                    (dv[0, :, g + 1, :], ot[64:128, hc:2 * hc]),
                    (dv[1, :, g + 1, :], ot[0:64, hc:2 * hc]),
                ]
                if split:
                    # tail strips: halves evacuated on ScalarE/DVE in parallel,
                    # stores chase each half; all on HWDGE queues so no
                    # GpSimd SWDGE drain delays the kernel end
                    nc.scalar.copy(out=ot[:, 0:hc], in_=pt[:, 0:hc])
                    nc.sync.dma_start(out=stores[0][0], in_=stores[0][1])
                    nc.vector.tensor_copy(out=ot[:, hc:ncol],
                                          in_=pt[:, hc:ncol])
                    nc.sync.dma_start(out=stores[1][0], in_=stores[1][1])
                    nc.scalar.dma_start(out=stores[2][0], in_=stores[2][1])
                else:
                    nc.scalar.copy(out=ot[:], in_=pt[:, 0:ncol])
                    for (dst, srcp), qc in zip(stores, STORE_Q):
                        qmap[qc].dma_start(out=dst, in_=srcp)

            pend_h2 = None    # (sv3, th3, s3, NP, r0, si) F strip awaiting H2+taps
            evac_fifo = []    # [(pt, R, NP, r0)]
            prev_sv = None    # (tile3, S, is_T): sv-valued buffer of prev strip
            prev_xb = None    # (xb3, nxr): previous strip's cast x tile

            has_indep = any(ind for _, _, ind in PLAN)

            # ---- strip geometry (plan order) ----
            geom = []
            r0n = 0
            for si, (R, folded, indep) in enumerate(PLAN):
                if indep:
                    r0 = H - R
                else:
                    r0 = r0n
                    r0n += R
                if indep:
                    x_lo = r0 - 2
                elif si == 0:
                    x_lo = r0
                else:
                    # ZOV=1: zero-overlap loads (min DMA, +2 small DVE ops);
                    # ZOV=0: 2-row halo re-load (min DVE ops, +1 MB DMA)
                    x_lo = r0 + 2 if ZOV else r0
                x_hi = min(H, r0 + R + 2)
                geom.append((r0, x_lo, x_hi))

            # ---- loads: SWDGE cast-DMA (f32 DRAM -> bf16 SBUF), all
            # pre-emitted so descriptor generation runs once up front and
            # every strip's data streams in as early as HBM bandwidth allows
            pre_xb = [None] * len(PLAN)
            if DMACAST:
                for si in range(len(PLAN)):
                    r0, x_lo, x_hi = geom[si]
                    nxr = x_hi - x_lo
                    xb = bpool.tile([128, nxr * W], BF16, tag="xb")
                    # all loads on the single Pool SWDGE ring: FIFO per ring,
                    # so strip 0 streams first at full bandwidth
                    nc.gpsimd.dma_start(out=xb[:], in_=x_pc[:, x_lo:x_hi, :])
                    pre_xb[si] = xb

            for si, (R, folded, indep) in enumerate(PLAN):
                first = si == 0
                r0, x_lo, x_hi = geom[si]
                # geometric bottom strip: handles row-127 reflect + zero row
                last = indep or (not has_indep and si == len(PLAN) - 1)
                r1 = r0 + R
                S = R + 2                     # s rows: positions [r0-1, r1+1)
                NP = R * W // 1024

                nxr = x_hi - x_lo
                if DMACAST:
                    xb = pre_xb[si]
                else:
                    xt = xpool.tile([128, nxr * W], F32, tag="xt")
                    xb = bpool.tile([128, nxr * W], BF16, tag="xb")
                    if first:
                        # split load + DVE cast (no ACT_TABLE_LOAD dependency)
                        # so the first strip's chain starts as early as can be
                        q0 = nc.scalar if L0Q == "a" else nc.sync
                        mid = nxr // 2
                        q0.dma_start(out=xt[:, 0:mid * W],
                                     in_=x_pc[:, x_lo:x_lo + mid, :])
                        q0.dma_start(out=xt[:, mid * W:],
                                     in_=x_pc[:, x_lo + mid:x_hi, :])
                        nc.vector.tensor_copy(out=xb[:, 0:mid * W],
                                              in_=xt[:, 0:mid * W])
                        nc.vector.tensor_copy(out=xb[:, mid * W:],
                                              in_=xt[:, mid * W:])
                    else:
                        nc.sync.dma_start(out=xt[:], in_=x_pc[:, x_lo:x_hi, :])
                        nc.scalar.copy(out=xb[:], in_=xt[:])
                xb3 = xb.rearrange("p (r w) -> p r w", w=W)

                # ---- V-box: sv[j] = x[j-1]+x[j]+x[j+1], reflect at 0/127 ----
                jlo = 0 if first else (r0 - 1 if indep else r0 + 1)
                jhi = H if last else r1 + 1
                nu = jhi - jlo
                jm_lo = max(jlo, x_lo + 1)
                jm_hi = min(jhi, x_hi - 1)
                ut = upool.tile([128, nu * W], BF16, tag="ut")
                u3 = ut.rearrange("p (r w) -> p r w", w=W)
                nc.vector.tensor_add(
                    out=u3[:, jm_lo - jlo:jm_hi - jlo, :],
                    in0=xb3[:, jm_lo - 1 - x_lo:jm_hi - 1 - x_lo, :],
                    in1=xb3[:, jm_lo + 1 - x_lo:jm_hi + 1 - x_lo, :])
                xtile = jm_lo > jlo and not first and not indep
                if xtile:
                    # boundary rows j in [jlo, jm_lo): x[j-1] from prev tile
                    pxb3, pnxr = prev_xb
                    nb = jm_lo - jlo   # == 2
                    nc.vector.tensor_add(
                        out=u3[:, 0:nb, :],
                        in0=pxb3[:, pnxr - nb:pnxr, :],
                        in1=xb3[:, jlo + 1 - x_lo:jm_lo + 1 - x_lo, :])
                if first:
                    nc.vector.tensor_scalar_mul(
                        out=u3[:, 0:1, :], in0=xb3[:, 1:2, :], scalar1=2.0)
                if last:
                    nc.vector.tensor_scalar_mul(
                        out=u3[:, nu - 1:nu, :],
                        in0=xb3[:, 126 - x_lo:127 - x_lo, :], scalar1=2.0)
                if not indep:
                    prev_xb = (xb3, nxr)

                st = spool.tile([128, S * WP], BF16, tag="st")
                s3 = st.rearrange("p (r c) -> p r c", c=WP)
                brow = jlo - (r0 - 1)
                if folded:
                    sv3 = None
                    v2out = lambda a, b: s3[:, brow + a:brow + b, 2:130]
                else:
                    svt = vpool.tile([128, S * W], BF16, tag="svt")
                    sv3 = svt.rearrange("p (r w) -> p r w", w=W)
                    v2out = lambda a, b: sv3[:, brow + a:brow + b, :]
                if xtile:
                    # sv[jlo]: center tap x[jlo] is prev tile's last row
                    nc.vector.tensor_add(
                        out=v2out(0, 1), in0=u3[:, 0:1, :],
                        in1=pxb3[:, pnxr - 1:pnxr, :])
                    nc.vector.tensor_add(
                        out=v2out(1, nu), in0=u3[:, 1:nu, :],
                        in1=xb3[:, jlo + 1 - x_lo:jhi - x_lo, :])
                else:
                    nc.vector.tensor_add(
                        out=v2out(0, nu), in0=u3[:, :, :],
                        in1=xb3[:, jlo - x_lo:jhi - x_lo, :])

                # ---- head rows [r0-1, r0+1): copy prev strip's last 2 sv rows
                if not first and not indep:
                    psv3, pS, p_folded = prev_sv
                    src = (psv3[:, pS - 2:pS, 2:130] if p_folded
                           else psv3[:, pS - 2:pS, :])
                    dst = (s3[:, 0:2, 2:130] if folded else sv3[:, 0:2, :])
                    if HEAD_SC:
                        nc.scalar.copy(out=dst, in_=src)
                    else:
                        nc.vector.tensor_copy(out=dst, in_=src)
                if not indep:
                    prev_sv = ((s3, S, True) if folded else (sv3, S, False))

                # ---- conv zero-pad rows at image top/bottom (before the
                # side-col fixups, which read all rows) ----
                if first:
                    nc.vector.memset(s3[:, 0:1, :], 0.0)
                if last:
                    nc.vector.memset(s3[:, S - 1:S, :], 0.0)

                # valid sv rows for th/H2: row 0 is the conv zero row on the
                # first strip, row S-1 on the geometric bottom strip
                h2lo = brow if first else 0
                h2hi = S - 1 if last else S
                if not folded:
                    # ---- H-box pair-add: th[w] = sv[w-1]+sv[w+1] ----
                    tht = tpool.tile([128, S * W], BF16, tag="tht")
                    th3 = tht.rearrange("p (r w) -> p r w", w=W)
                    hp = max(h2lo, min(h2hi, h2lo + H1_POOL_ROWS))
                    if hp > h2lo:
                        nc.gpsimd.tensor_add(out=th3[:, h2lo:hp, 1:127],
                                             in0=sv3[:, h2lo:hp, 0:126],
                                             in1=sv3[:, h2lo:hp, 2:128])
                    if hp < h2hi:
                        nc.vector.tensor_add(out=th3[:, hp:h2hi, 1:127],
                                             in0=sv3[:, hp:h2hi, 0:126],
                                             in1=sv3[:, hp:h2hi, 2:128])
                    if EDGE_SC:
                        nc.scalar.mul(th3[:, h2lo:h2hi, 0:1],
                                      sv3[:, h2lo:h2hi, 1:2], 2.0)
                        nc.scalar.mul(th3[:, h2lo:h2hi, 127:128],
                                      sv3[:, h2lo:h2hi, 126:127], 2.0)
                    else:
                        nc.vector.tensor_scalar_mul(
                            out=th3[:, h2lo:h2hi, 0:1],
                            in0=sv3[:, h2lo:h2hi, 1:2], scalar1=2.0)
                        nc.vector.tensor_scalar_mul(
                            out=th3[:, h2lo:h2hi, 127:128],
                            in0=sv3[:, h2lo:h2hi, 126:127], scalar1=2.0)
                    nc.vector.memset(s3[:, :, 0:2], 0.0)
                    nc.vector.memset(s3[:, :, 130:132], 0.0)
                else:
                    # ---- folded side cols: col c reads sv[c-2] ----
                    nc.vector.tensor_copy(out=s3[:, :, 1:2], in_=s3[:, :, 3:4])
                    nc.vector.tensor_copy(out=s3[:, :, 130:131],
                                          in_=s3[:, :, 128:129])
                    nc.vector.scalar_tensor_tensor(
                        out=s3[:, :, 0:1], in0=s3[:, :, 2:3], scalar=-1.0,
                        in1=s3[:, :, 3:4], op0=mybir.AluOpType.mult,
                        op1=mybir.AluOpType.subtract)
                    nc.vector.scalar_tensor_tensor(
                        out=s3[:, :, 131:132], in0=s3[:, :, 129:130],
                        scalar=-1.0, in1=s3[:, :, 128:129],
                        op0=mybir.AluOpType.mult,
                        op1=mybir.AluOpType.subtract)

                if folded:
                    for h, off in enumerate(range(0, R, CHUNK)):
                        rows = min(CHUNK, R - off)
                        cnp = rows * W // 1024
                        pt = emit_taps(s3, True, cnp, r0 + off, 10 * si + h,
                                       pt=warm_pt if (first and h == 0)
                                       else None, row_off=off)
                        evac_fifo.append((pt, rows, cnp, r0 + off))
                        while len(evac_fifo) > 1:
                            emit_evac_store(*evac_fifo.pop(0))

                def emit_h2_taps(dsv3, dth3, ds3, dR, dr0, dsi, dlo, dhi):
                    nc.vector.tensor_add(out=ds3[:, dlo:dhi, 2:130],
                                         in0=dth3[:, dlo:dhi, 0:128],
                                         in1=dsv3[:, dlo:dhi, 0:128])
                    # taps/evac/stores in CHUNK-row blocks (psum slots)
                    for h, off in enumerate(range(0, dR, CHUNK)):
                        rows = min(CHUNK, dR - off)
                        cnp = rows * W // 1024
                        pt = emit_taps(ds3, False, cnp, dr0 + off,
                                       10 * dsi + h, row_off=off)
                        evac_fifo.append((pt, rows, cnp, dr0 + off))
                        while len(evac_fifo) > 1:
                            emit_evac_store(*evac_fifo.pop(0))

                # ---- deferred H2 + taps of the previous F strip (emitted
                # after a folded strip's taps so those hide under the H2) ----
                if pend_h2 is not None:
                    emit_h2_taps(*pend_h2)
                    pend_h2 = None

                if folded:
                    pass
                elif H1_POOL_ROWS == 0 and not (si + 1 < len(PLAN)
                                                and PLAN[si + 1][1]):
                    # H1 all on DVE: no cross-engine stall risk, emit inline
                    emit_h2_taps(sv3, th3, s3, R, r0, si, h2lo, h2hi)
                else:
                    # last F strip (or H1_POOL_ROWS mode): defer H2+taps
                    pend_h2 = (sv3, th3, s3, R, r0, si, h2lo, h2hi)

                while len(evac_fifo) > 1:
                    emit_evac_store(*evac_fifo.pop(0))

            if pend_h2 is not None:
                emit_h2_taps(*pend_h2)
            while evac_fifo:
                emit_evac_store(*evac_fifo.pop(0), split=True)

    nc.compile()
    return nc


_CACHE = {}


def _get_nc():
    if "nc" not in _CACHE:
        _CACHE["nc"] = build()
    return _CACHE["nc"]


def kernel(x: np.ndarray, W: np.ndarray, trace: bool = False):
    x = np.asarray(x, dtype=np.float32)
    wt = _host_weights(np.asarray(W, dtype=np.float32))
    nc = _get_nc()
    core_ids = list(range(N_CORES))
    in_maps = [
        {"x": np.ascontiguousarray(x[BPC * i:BPC * (i + 1)]), "wt": wt}
        for i in core_ids
    ]
    res = run_bass_kernel_spmd(nc, in_maps, core_ids, trace=trace)
    out = np.concatenate(
        [res.results[i]["out"].astype(np.float32) for i in core_ids], axis=0)
    if trace:
        kernel.last_exec_time_ns = res.exec_time_ns
        kernel.last_res = res
    return out


kernel.last_exec_time_ns = None



# revision 21
# speedup vs baseline: 1.0639x; 1.0243x over previous
"""Trainium2 kernel for Conv2d_cd (central-difference conv, 3x3, theta=0.7).

Reference math:
    s = sum of 9 shifted views of reflect-padded x  (= 3x3 box filter, reflect pad)
    out = conv3x3_zeropad(s, W) - theta * conv1x1(s, W.sum((2,3)))
        = conv3x3_zeropad(s, W')     with W'[:,:,1,1] -= theta * W.sum((2,3))

Strategy (per NeuronCore, 8 cores data-parallel over batch 16 -> 2 images/core):
  - images stacked on SBUF partition halves: partitions 0:64 = img0 ch, 64:128 = img1 ch
  - H strips; per strip: cast x->bf16 on ScalarE, V-box (2 bf16 2x adds on DVE).
    F strips: H-box pair-add split GpSimd/DVE, center-add (2x) on DVE deferred one
    strip (so the GpSimd part never stalls the DVE queue), then 9 conv taps.
    T strips (ends): H-box folded into 15 taps.  Strips share their 2 boundary s
    rows via a tiny copy instead of recomputation.
  - taps as K=64/M=64 matmuls packed 4-concurrent in PE quadrants into one
    [128,2048] PSUM tile; chunk->row permutation [0,2,1,3] makes each store
    destination 8 contiguous DRAM rows.  Evacuation: single ScalarE copy to fp16
    (output stored as fp16, widened to fp32 on host; rel-err budget 2e-2).
"""

import os

import numpy as np
import ml_dtypes

import concourse.bass as bass
import concourse.bacc as bacc
import concourse.mybir as mybir
from concourse.tile import TileContext
from concourse.bass_utils import run_bass_kernel_spmd

THETA = 0.7
N_CORES = 8
B, C, H, W = 16, 64, 128, 128
BPC = B // N_CORES          # images per core = 2
WP = W + 4                  # padded width of s tiles (132)
F32 = mybir.dt.float32
BF16 = mybir.dt.bfloat16
F16 = mybir.dt.float16


def _parse_plan():
    # kinds: T = folded (15-tap), F = unfolded (9-tap + H on DVE),
    # I = independent folded bottom strip (rows [128-R, 128), computes its
    # own boundary rows; lets the bottom taps run early instead of last)
    txt = os.environ.get("K_PLAN", "F8,J8,F16,F16,F16,F16,F16,F16,F8,F8")
    plan = []
    for item in txt.split(","):
        item = item.strip()
        plan.append((int(item[1:]), item[0] in "TI", item[0] in "IJ"))
    assert sum(r for r, _, _ in plan) == H, plan
    return plan


PLAN = _parse_plan()
OUT16 = os.environ.get("K_OUT16", "1") == "1"   # store output as fp16
H1_POOL_ROWS = int(os.environ.get("K_H1_POOL_ROWS", "0"))  # of S=18 on GpSimd
STORE_Q = os.environ.get("K_STORE_Q", "ssg")  # a=scalar g=gpsimd s=sync v=vector
DMACAST = os.environ.get("K_DMACAST", "0") == "1"  # f32->bf16 cast in SWDGE DMA
XBUFS = int(os.environ.get("K_XBUFS", str(len(PLAN) if DMACAST else 4)))
SBUFS = int(os.environ.get("K_SBUFS", "4"))
OBUFS = int(os.environ.get("K_OBUFS", "3"))
PBUFS = int(os.environ.get("K_PBUFS", "4"))    # psum rotation slots (2 banks ea)
CHUNK = int(os.environ.get("K_CHUNK", "8"))    # tap/evac chunk rows
ODT = F16 if OUT16 else F32
EDGE_SC = os.environ.get("K_EDGE_SC", "1" if DMACAST else "0") == "1"
ZOV = os.environ.get("K_ZOV", "0") == "1"
HEAD_SC = os.environ.get("K_HEAD_SC", "1" if DMACAST else "0") == "1"
L0Q = os.environ.get("K_L0Q", "s")  # first-strip load ring: s=sync a=scalar


def _host_weights(Wnp: np.ndarray):
    """W' and packed tap weights: wt[p, t*C + co], taps 0..8 = w9 (ky*3+kx),
    taps 9..23 = w15 (ky*5+tx); both partition halves identical."""
    Wp = Wnp.astype(np.float64).copy()
    Wp[:, :, 1, 1] -= THETA * Wnp.astype(np.float64).sum(axis=(2, 3))
    w9 = Wp.transpose(1, 2, 3, 0).reshape(C, 9, C)
    w15 = np.zeros((C, 3, 5, C), np.float64)
    for ky in range(3):
        for tx in range(5):
            for kx in range(max(0, tx - 2), min(2, tx) + 1):
                w15[:, ky, tx, :] += Wp[:, :, ky, kx].T  # [ci, co]
    wt = np.concatenate([w9.reshape(C, 9 * C), w15.reshape(C, 15 * C)], axis=1)
    wt = np.concatenate([wt, wt], axis=0)
    return np.ascontiguousarray(wt.astype(ml_dtypes.bfloat16))


def build():
    nc = bacc.Bacc("TRN2", target_bir_lowering=False, debug=False,
                   num_devices=N_CORES)
    x_d = nc.declare_dram_parameter("x", [BPC, C, H, W], F32, isOutput=False)
    wt_d = nc.declare_dram_parameter("wt", [128, 24 * C], BF16, isOutput=False)
    out_d = nc.declare_dram_parameter("out", [BPC, C, H, W], ODT, isOutput=True)

    x_pc = x_d.rearrange("i c h w -> (i c) h w")
    # store views: 8 (or 4) contiguous rows per (img, ch) descriptor
    out8 = out_d.rearrange("i c (g8 eight) w -> i c g8 (eight w)", eight=8)
    out4 = out_d.rearrange("i c (g4 four) w -> i c g4 (four w)", four=4)
    out8pc = out_d.rearrange("i c (g8 eight) w -> (i c) g8 (eight w)", eight=8)
    out4pc = out_d.rearrange("i c (g4 four) w -> (i c) g4 (four w)", four=4)

    pam = os.environ.get("K_POOL_MODE", "queue")
    with TileContext(nc, pool_alloc_mode=pam) as tc:
        with (
            tc.tile_pool(name="wpool", bufs=1) as wpool,
            tc.tile_pool(name="warmpool", bufs=1) as warmpool,
            tc.tile_pool(name="xpool", bufs=XBUFS) as xpool,
            tc.tile_pool(name="bpool", bufs=XBUFS) as bpool,
            tc.tile_pool(name="upool", bufs=2) as upool,
            tc.tile_pool(name="vpool", bufs=2) as vpool,
            tc.tile_pool(name="tpool", bufs=2) as tpool,
            tc.tile_pool(name="spool", bufs=SBUFS) as spool,
            tc.tile_pool(name="opool", bufs=OBUFS) as opool,
            tc.tile_pool(name="psum", bufs=PBUFS, space="PSUM") as ppool,
        ):
            wt_sb = wpool.tile([128, 24 * C], BF16)
            (nc.sync if L0Q == "a" else nc.scalar).dma_start(
                out=wt_sb[:], in_=wt_d[:])

            # PE p-state warmup: dep-free matmuls keep the tensor engine
            # busy through the load phase so the first real taps run at
            # full clock (ramp needs ~3us of sustained execution)
            NWARM = int(os.environ.get("K_WARM", "14"))
            warm_pt = None
            if NWARM:
                wrm = warmpool.tile([128, 512], BF16)
                nc.vector.memset(wrm[:], 0.0)
                NP0 = PLAN[0][0] * W // 1024
                warm_pt = ppool.tile([128, 1024 * NP0], F32, tag="ps",
                                     name="warm_ps")
                for _ in range(NWARM):
                    nc.tensor.matmul(warm_pt[0:64, 0:512], wrm[0:64, 0:64],
                                     wrm[0:64, :], start=True, stop=True,
                                     skip_group_check=True)

            qmap = {"a": nc.scalar, "g": nc.gpsimd, "s": nc.sync,
                    "v": nc.vector}

            def emit_taps(s3, folded, NP, r0, si, pt=None, row_off=0):
                # psum col block j holds output rows [4j, 4j+4) (row-major).
                # Per (pair, tap): 4 quadrant matmuls on blocks {p, p+NP}:
                # img i -> half i on block p, half 1-i on block p+NP, so ot
                # halves end up [img rows 0..R/2 | img rows R/2..R] contiguous.
                if pt is None:
                    pt = ppool.tile([128, 1024 * NP], F32, tag="ps",
                                    name=f"ps{si}")
                ntap = 15 if folded else 9
                tap0 = 9 if folded else 0
                nkx = 5 if folded else 3
                cofs = 0 if folded else 1
                for p in range(NP):
                    for t in range(ntap):
                        ky, kx = t // nkx, t % nkx
                        tw = tap0 + t
                        for (i, j, pbase) in ((0, p, 0), (1, p, 64),
                                              (0, p + NP, 64), (1, p + NP, 0)):
                            rhs = s3[64 * i:64 * i + 64,
                                     row_off + 4 * j + ky:
                                     row_off + 4 * j + ky + 4,
                                     kx + cofs:kx + cofs + 128]
                            nc.tensor.matmul(
                                pt[pbase:pbase + 64, 512 * j:512 * j + 512],
                                wt_sb[64 * i:64 * i + 64, tw * C:(tw + 1) * C],
                                rhs,
                                start=(t == 0), stop=(t == ntap - 1),
                                skip_group_check=True,
                            )
                return pt

            def emit_evac_store(pt, R, NP, r0, split=False):
                ncol = R * W
                hc = 512 * NP   # half of the strip's rows
                ot = opool.tile([128, ncol], ODT, tag="ot", name=f"ot{r0}")
                if NP == 2:
                    dv, dvpc, g = out8, out8pc, r0 // 8
                else:
                    dv, dvpc, g = out4, out4pc, r0 // 4
                stores = [
                    (dvpc[:, g + 0, :], ot[:, 0:hc]),
                    (dv[0, :, g + 1, :], ot[64:128, hc:2 * hc]),
                    (dv[1, :, g + 1, :], ot[0:64, hc:2 * hc]),
                ]
                if split:
                    # tail strips: halves evacuated on ScalarE/DVE in parallel,
                    # stores chase each half; all on HWDGE queues so no
                    # GpSimd SWDGE drain delays the kernel end
                    nc.scalar.copy(out=ot[:, 0:hc], in_=pt[:, 0:hc])
                    nc.sync.dma_start(out=stores[0][0], in_=stores[0][1])
                    nc.vector.tensor_copy(out=ot[:, hc:ncol],
                                          in_=pt[:, hc:ncol])
                    nc.sync.dma_start(out=stores[1][0], in_=stores[1][1])
                    nc.scalar.dma_start(out=stores[2][0], in_=stores[2][1])
                else:
                    nc.scalar.copy(out=ot[:], in_=pt[:, 0:ncol])
                    for (dst, srcp), qc in zip(stores, STORE_Q):
                        qmap[qc].dma_start(out=dst, in_=srcp)

            pend_h2 = None    # (sv3, th3, s3, NP, r0, si) F strip awaiting H2+taps
            evac_fifo = []    # [(pt, R, NP, r0)]
            prev_sv = None    # (tile3, S, is_T): sv-valued buffer of prev strip
            prev_xb = None    # (xb3, nxr): previous strip's cast x tile

            has_indep = any(ind for _, _, ind in PLAN)

            # ---- strip geometry (plan order) ----
            geom = []
            r0n = 0
            for si, (R, folded, indep) in enumerate(PLAN):
                if indep:
                    r0 = H - R
                else:
                    r0 = r0n
                    r0n += R
                if indep:
                    x_lo = r0 - 2
                elif si == 0:
                    x_lo = r0
                else:
                    # ZOV=1: zero-overlap loads (min DMA, +2 small DVE ops);
                    # ZOV=0: 2-row halo re-load (min DVE ops, +1 MB DMA)
                    x_lo = r0 + 2 if ZOV else r0
                x_hi = min(H, r0 + R + 2)
                geom.append((r0, x_lo, x_hi))

            # ---- loads: SWDGE cast-DMA (f32 DRAM -> bf16 SBUF), all
            # pre-emitted so descriptor generation runs once up front and
            # every strip's data streams in as early as HBM bandwidth allows
            pre_xb = [None] * len(PLAN)
            if DMACAST:
                for si in range(len(PLAN)):
                    r0, x_lo, x_hi = geom[si]
                    nxr = x_hi - x_lo
                    xb = bpool.tile([128, nxr * W], BF16, tag="xb")
                    # all loads on the single Pool SWDGE ring: FIFO per ring,
                    # so strip 0 streams first at full bandwidth
                    nc.gpsimd.dma_start(out=xb[:], in_=x_pc[:, x_lo:x_hi, :])
                    pre_xb[si] = xb

            for si, (R, folded, indep) in enumerate(PLAN):
                first = si == 0
                r0, x_lo, x_hi = geom[si]
                # geometric bottom strip: handles row-127 reflect + zero row
                last = indep or (not has_indep and si == len(PLAN) - 1)
                r1 = r0 + R
                S = R + 2                     # s rows: positions [r0-1, r1+1)
                NP = R * W // 1024

                nxr = x_hi - x_lo
                if DMACAST:
                    xb = pre_xb[si]
                else:
                    xt = xpool.tile([128, nxr * W], F32, tag="xt")
                    xb = bpool.tile([128, nxr * W], BF16, tag="xb")
                    if first:
                        # split load + DVE cast (no ACT_TABLE_LOAD dependency)
                        # so the first strip's chain starts as early as can be
                        q0 = nc.scalar if L0Q == "a" else nc.sync
                        mid = nxr // 2
                        q0.dma_start(out=xt[:, 0:mid * W],
                                     in_=x_pc[:, x_lo:x_lo + mid, :])
                        q0.dma_start(out=xt[:, mid * W:],
                                     in_=x_pc[:, x_lo + mid:x_hi, :])
                        nc.vector.tensor_copy(out=xb[:, 0:mid * W],
                                              in_=xt[:, 0:mid * W])
                        nc.vector.tensor_copy(out=xb[:, mid * W:],
                                              in_=xt[:, mid * W:])
                    else:
                        nc.sync.dma_start(out=xt[:], in_=x_pc[:, x_lo:x_hi, :])
                        nc.scalar.copy(out=xb[:], in_=xt[:])
                xb3 = xb.rearrange("p (r w) -> p r w", w=W)

                # ---- V-box: sv[j] = x[j-1]+x[j]+x[j+1], reflect at 0/127 ----
                jlo = 0 if first else (r0 - 1 if indep else r0 + 1)
                jhi = H if last else r1 + 1
                nu = jhi - jlo
                jm_lo = max(jlo, x_lo + 1)
                jm_hi = min(jhi, x_hi - 1)
                ut = upool.tile([128, nu * W], BF16, tag="ut")
                u3 = ut.rearrange("p (r w) -> p r w", w=W)
                nc.vector.tensor_add(
                    out=u3[:, jm_lo - jlo:jm_hi - jlo, :],
                    in0=xb3[:, jm_lo - 1 - x_lo:jm_hi - 1 - x_lo, :],
                    in1=xb3[:, jm_lo + 1 - x_lo:jm_hi + 1 - x_lo, :])
                xtile = jm_lo > jlo and not first and not indep
                if xtile:
                    # boundary rows j in [jlo, jm_lo): x[j-1] from prev tile
                    pxb3, pnxr = prev_xb
                    nb = jm_lo - jlo   # == 2
                    nc.vector.tensor_add(
                        out=u3[:, 0:nb, :],
                        in0=pxb3[:, pnxr - nb:pnxr, :],
                        in1=xb3[:, jlo + 1 - x_lo:jm_lo + 1 - x_lo, :])
                if first:
                    nc.vector.tensor_scalar_mul(
                        out=u3[:, 0:1, :], in0=xb3[:, 1:2, :], scalar1=2.0)
                if last:
                    nc.vector.tensor_scalar_mul(
                        out=u3[:, nu - 1:nu, :],
                        in0=xb3[:, 126 - x_lo:127 - x_lo, :], scalar1=2.0)
                if not indep:
                    prev_xb = (xb3, nxr)

                st = spool.tile([128, S * WP], BF16, tag="st")
                s3 = st.rearrange("p (r c) -> p r c", c=WP)
                brow = jlo - (r0 - 1)
                if folded:
                    sv3 = None
                    v2out = lambda a, b: s3[:, brow + a:brow + b, 2:130]
                else:
                    svt = vpool.tile([128, S * W], BF16, tag="svt")
                    sv3 = svt.rearrange("p (r w) -> p r w", w=W)
                    v2out = lambda a, b: sv3[:, brow + a:brow + b, :]
                if xtile:
                    # sv[jlo]: center tap x[jlo] is prev tile's last row
                    nc.vector.tensor_add(
                        out=v2out(0, 1), in0=u3[:, 0:1, :],
                        in1=pxb3[:, pnxr - 1:pnxr, :])
                    nc.vector.tensor_add(
                        out=v2out(1, nu), in0=u3[:, 1:nu, :],
                        in1=xb3[:, jlo + 1 - x_lo:jhi - x_lo, :])
                else:
                    nc.vector.tensor_add(
                        out=v2out(0, nu), in0=u3[:, :, :],
                        in1=xb3[:, jlo - x_lo:jhi - x_lo, :])

                # ---- head rows [r0-1, r0+1): copy prev strip's last 2 sv rows
                if not first and not indep:
                    psv3, pS, p_folded = prev_sv
                    src = (psv3[:, pS - 2:pS, 2:130] if p_folded
                           else psv3[:, pS - 2:pS, :])
                    dst = (s3[:, 0:2, 2:130] if folded else sv3[:, 0:2, :])
                    if HEAD_SC:
                        nc.scalar.copy(out=dst, in_=src)
                    else:
                        nc.vector.tensor_copy(out=dst, in_=src)
                if not indep:
                    prev_sv = ((s3, S, True) if folded else (sv3, S, False))

                # ---- conv zero-pad rows at image top/bottom (before the
                # side-col fixups, which read all rows) ----
                if first:
                    nc.vector.memset(s3[:, 0:1, :], 0.0)
                if last:
                    nc.vector.memset(s3[:, S - 1:S, :], 0.0)

                # valid sv rows for th/H2: row 0 is the conv zero row on the
                # first strip, row S-1 on the geometric bottom strip
                h2lo = brow if first else 0
                h2hi = S - 1 if last else S
                if not folded:
                    # ---- H-box pair-add: th[w] = sv[w-1]+sv[w+1] ----
                    tht = tpool.tile([128, S * W], BF16, tag="tht")
                    th3 = tht.rearrange("p (r w) -> p r w", w=W)
                    hp = max(h2lo, min(h2hi, h2lo + H1_POOL_ROWS))
                    if hp > h2lo:
                        nc.gpsimd.tensor_add(out=th3[:, h2lo:hp, 1:127],
                                             in0=sv3[:, h2lo:hp, 0:126],
                                             in1=sv3[:, h2lo:hp, 2:128])
                    if hp < h2hi:
                        nc.vector.tensor_add(out=th3[:, hp:h2hi, 1:127],
                                             in0=sv3[:, hp:h2hi, 0:126],
                                             in1=sv3[:, hp:h2hi, 2:128])
                    if EDGE_SC:
                        nc.scalar.mul(th3[:, h2lo:h2hi, 0:1],
                                      sv3[:, h2lo:h2hi, 1:2], 2.0)
                        nc.scalar.mul(th3[:, h2lo:h2hi, 127:128],
                                      sv3[:, h2lo:h2hi, 126:127], 2.0)
                    else:
                        nc.vector.tensor_scalar_mul(
                            out=th3[:, h2lo:h2hi, 0:1],
                            in0=sv3[:, h2lo:h2hi, 1:2], scalar1=2.0)
                        nc.vector.tensor_scalar_mul(
                            out=th3[:, h2lo:h2hi, 127:128],
                            in0=sv3[:, h2lo:h2hi, 126:127], scalar1=2.0)
                    nc.vector.memset(s3[:, :, 0:2], 0.0)
                    nc.vector.memset(s3[:, :, 130:132], 0.0)
                else:
                    # ---- folded side cols: col c reads sv[c-2] ----
                    nc.vector.tensor_copy(out=s3[:, :, 1:2], in_=s3[:, :, 3:4])
                    nc.vector.tensor_copy(out=s3[:, :, 130:131],
                                          in_=s3[:, :, 128:129])
                    nc.vector.scalar_tensor_tensor(
                        out=s3[:, :, 0:1], in0=s3[:, :, 2:3], scalar=-1.0,
                        in1=s3[:, :, 3:4], op0=mybir.AluOpType.mult,
                        op1=mybir.AluOpType.subtract)
                    nc.vector.scalar_tensor_tensor(
                        out=s3[:, :, 131:132], in0=s3[:, :, 129:130],
                        scalar=-1.0, in1=s3[:, :, 128:129],
                        op0=mybir.AluOpType.mult,
                        op1=mybir.AluOpType.subtract)

                if folded:
                    for h, off in enumerate(range(0, R, CHUNK)):
                        rows = min(CHUNK, R - off)
                        cnp = rows * W // 1024
                        pt = emit_taps(s3, True, cnp, r0 + off, 10 * si + h,
                                       pt=warm_pt if (first and h == 0)
                                       else None, row_off=off)
                        evac_fifo.append((pt, rows, cnp, r0 + off))
                        while len(evac_fifo) > 1:
                            emit_evac_store(*evac_fifo.pop(0))

                def emit_h2_taps(dsv3, dth3, ds3, dR, dr0, dsi, dlo, dhi):
                    nc.vector.tensor_add(out=ds3[:, dlo:dhi, 2:130],
                                         in0=dth3[:, dlo:dhi, 0:128],
                                         in1=dsv3[:, dlo:dhi, 0:128])
                    # taps/evac/stores in CHUNK-row blocks (psum slots)
                    for h, off in enumerate(range(0, dR, CHUNK)):
                        rows = min(CHUNK, dR - off)
                        cnp = rows * W // 1024
                        pt = emit_taps(ds3, False, cnp, dr0 + off,
                                       10 * dsi + h, row_off=off)
                        evac_fifo.append((pt, rows, cnp, dr0 + off))
                        while len(evac_fifo) > 1:
                            emit_evac_store(*evac_fifo.pop(0))

                # ---- deferred H2 + taps of the previous F strip (emitted
                # after a folded strip's taps so those hide under the H2) ----
                if pend_h2 is not None:
                    emit_h2_taps(*pend_h2)
                    pend_h2 = None

                if folded:
                    pass
                elif H1_POOL_ROWS == 0 and not (si + 1 < len(PLAN)
                                                and PLAN[si + 1][1]):
                    # H1 all on DVE: no cross-engine stall risk, emit inline
                    emit_h2_taps(sv3, th3, s3, R, r0, si, h2lo, h2hi)
                else:
                    # last F strip (or H1_POOL_ROWS mode): defer H2+taps
                    pend_h2 = (sv3, th3, s3, R, r0, si, h2lo, h2hi)

                while len(evac_fifo) > 1:
                    emit_evac_store(*evac_fifo.pop(0))

            if pend_h2 is not None:
                emit_h2_taps(*pend_h2)
            while evac_fifo:
                emit_evac_store(*evac_fifo.pop(0), split=True)

    nc.compile()
    return nc


_CACHE = {}


def _get_nc():
    if "nc" not in _CACHE:
        _CACHE["nc"] = build()
    return _CACHE["nc"]


def kernel(x: np.ndarray, W: np.ndarray, trace: bool = False):
    x = np.asarray(x, dtype=np.float32)
    wt = _host_weights(np.asarray(W, dtype=np.float32))
    nc = _get_nc()
    core_ids = list(range(N_CORES))
    in_maps = [
        {"x": np.ascontiguousarray(x[BPC * i:BPC * (i + 1)]), "wt": wt}
        for i in core_ids
    ]
    res = run_bass_kernel_spmd(nc, in_maps, core_ids, trace=trace)
    out = np.concatenate(
        [res.results[i]["out"].astype(np.float32) for i in core_ids], axis=0)
    if trace:
        kernel.last_exec_time_ns = res.exec_time_ns
        kernel.last_res = res
    return out


kernel.last_exec_time_ns = None



# revision 23
# speedup vs baseline: 1.0872x; 1.0220x over previous
"""Trainium2 kernel for Conv2d_cd (central-difference conv, 3x3, theta=0.7).

Reference math:
    s = sum of 9 shifted views of reflect-padded x  (= 3x3 box filter, reflect pad)
    out = conv3x3_zeropad(s, W) - theta * conv1x1(s, W.sum((2,3)))
        = conv3x3_zeropad(s, W')     with W'[:,:,1,1] -= theta * W.sum((2,3))

Strategy (per NeuronCore, 8 cores data-parallel over batch 16 -> 2 images/core):
  - images stacked on SBUF partition halves: partitions 0:64 = img0 ch, 64:128 = img1 ch
  - H strips; per strip: cast x->bf16 on ScalarE, V-box (2 bf16 2x adds on DVE).
    F strips: H-box pair-add split GpSimd/DVE, center-add (2x) on DVE deferred one
    strip (so the GpSimd part never stalls the DVE queue), then 9 conv taps.
    T strips (ends): H-box folded into 15 taps.  Strips share their 2 boundary s
    rows via a tiny copy instead of recomputation.
  - taps as K=64/M=64 matmuls packed 4-concurrent in PE quadrants into one
    [128,2048] PSUM tile; chunk->row permutation [0,2,1,3] makes each store
    destination 8 contiguous DRAM rows.  Evacuation: single ScalarE copy to fp16
    (output stored as fp16, widened to fp32 on host; rel-err budget 2e-2).
"""

import os

import numpy as np
import ml_dtypes

import concourse.bass as bass
import concourse.bacc as bacc
import concourse.mybir as mybir
from concourse.tile import TileContext
from concourse.bass_utils import run_bass_kernel_spmd

THETA = 0.7
N_CORES = 8
B, C, H, W = 16, 64, 128, 128
BPC = B // N_CORES          # images per core = 2
WP = W + 4                  # padded width of s tiles (132)
F32 = mybir.dt.float32
BF16 = mybir.dt.bfloat16
F16 = mybir.dt.float16


def _parse_plan():
    # kinds: T = folded (15-tap), F = unfolded (9-tap + H on DVE),
    # I = independent folded bottom strip (rows [128-R, 128), computes its
    # own boundary rows; lets the bottom taps run early instead of last)
    txt = os.environ.get("K_PLAN", "F8,J8,F16,F16,F16,F16,F16,F16,F8,F8")
    plan = []
    for item in txt.split(","):
        item = item.strip()
        plan.append((int(item[1:]), item[0] in "TI", item[0] in "IJ"))
    assert sum(r for r, _, _ in plan) == H, plan
    return plan


PLAN = _parse_plan()
OUT16 = os.environ.get("K_OUT16", "1") == "1"   # store output as fp16
H1_POOL_ROWS = int(os.environ.get("K_H1_POOL_ROWS", "0"))  # of S=18 on GpSimd
STORE_Q = os.environ.get("K_STORE_Q", "ssg")  # a=scalar g=gpsimd s=sync v=vector
DMACAST = os.environ.get("K_DMACAST", "0") == "1"  # f32->bf16 cast in SWDGE DMA
XBUFS = int(os.environ.get("K_XBUFS", str(len(PLAN) if DMACAST else 4)))
SBUFS = int(os.environ.get("K_SBUFS", "4"))
OBUFS = int(os.environ.get("K_OBUFS", "3"))
PBUFS = int(os.environ.get("K_PBUFS", "4"))    # psum rotation slots (2 banks ea)
CHUNK = int(os.environ.get("K_CHUNK", "8"))    # tap/evac chunk rows
ODT = F16 if OUT16 else F32
EDGE_SC = os.environ.get("K_EDGE_SC", "1" if DMACAST else "0") == "1"
ZOV = os.environ.get("K_ZOV", "0") == "1"
HEAD_SC = os.environ.get("K_HEAD_SC", "1" if DMACAST else "0") == "1"
L0Q = os.environ.get("K_L0Q", "s")  # first-strip load ring: s=sync a=scalar


def _host_weights(Wnp: np.ndarray):
    """W' and packed tap weights: wt[p, t*C + co], taps 0..8 = w9 (ky*3+kx),
    taps 9..23 = w15 (ky*5+tx); both partition halves identical."""
    Wp = Wnp.astype(np.float64).copy()
    Wp[:, :, 1, 1] -= THETA * Wnp.astype(np.float64).sum(axis=(2, 3))
    w9 = Wp.transpose(1, 2, 3, 0).reshape(C, 9, C)
    w15 = np.zeros((C, 3, 5, C), np.float64)
    for ky in range(3):
        for tx in range(5):
            for kx in range(max(0, tx - 2), min(2, tx) + 1):
                w15[:, ky, tx, :] += Wp[:, :, ky, kx].T  # [ci, co]
    wt = np.concatenate([w9.reshape(C, 9 * C), w15.reshape(C, 15 * C)], axis=1)
    wt = np.concatenate([wt, wt], axis=0)
    return np.ascontiguousarray(wt.astype(ml_dtypes.bfloat16))


def build():
    nc = bacc.Bacc("TRN2", target_bir_lowering=False, debug=False,
                   num_devices=N_CORES)
    x_d = nc.declare_dram_parameter("x", [BPC, C, H, W], F32, isOutput=False)
    wt_d = nc.declare_dram_parameter("wt", [128, 24 * C], BF16, isOutput=False)
    out_d = nc.declare_dram_parameter("out", [BPC, C, H, W], ODT, isOutput=True)

    x_pc = x_d.rearrange("i c h w -> (i c) h w")
    # store views: 8 (or 4) contiguous rows per (img, ch) descriptor
    out8 = out_d.rearrange("i c (g8 eight) w -> i c g8 (eight w)", eight=8)
    out4 = out_d.rearrange("i c (g4 four) w -> i c g4 (four w)", four=4)
    out8pc = out_d.rearrange("i c (g8 eight) w -> (i c) g8 (eight w)", eight=8)
    out4pc = out_d.rearrange("i c (g4 four) w -> (i c) g4 (four w)", four=4)

    pam = os.environ.get("K_POOL_MODE", "queue")
    with TileContext(nc, pool_alloc_mode=pam) as tc:
        with (
            tc.tile_pool(name="wpool", bufs=1) as wpool,
            tc.tile_pool(name="warmpool", bufs=1) as warmpool,
            tc.tile_pool(name="xpool", bufs=XBUFS) as xpool,
            tc.tile_pool(name="bpool", bufs=XBUFS) as bpool,
            tc.tile_pool(name="upool", bufs=2) as upool,
            tc.tile_pool(name="vpool", bufs=2) as vpool,
            tc.tile_pool(name="tpool", bufs=2) as tpool,
            tc.tile_pool(name="spool", bufs=SBUFS) as spool,
            tc.tile_pool(name="opool", bufs=OBUFS) as opool,
            tc.tile_pool(name="psum", bufs=PBUFS, space="PSUM") as ppool,
        ):
            wt_sb = wpool.tile([128, 24 * C], BF16)
            (nc.sync if L0Q in "ab" else nc.scalar).dma_start(
                out=wt_sb[:], in_=wt_d[:])

            # PE p-state warmup: dep-free matmuls keep the tensor engine
            # busy through the load phase so the first real taps run at
            # full clock (ramp needs ~3us of sustained execution)
            NWARM = int(os.environ.get("K_WARM", "14"))
            warm_pt = None
            if NWARM:
                wrm = warmpool.tile([128, 512], BF16)
                nc.vector.memset(wrm[:], 0.0)
                NP0 = PLAN[0][0] * W // 1024
                warm_pt = ppool.tile([128, 1024 * NP0], F32, tag="ps",
                                     name="warm_ps")
                for _ in range(NWARM):
                    nc.tensor.matmul(warm_pt[0:64, 0:512], wrm[0:64, 0:64],
                                     wrm[0:64, :], start=True, stop=True,
                                     skip_group_check=True)

            qmap = {"a": nc.scalar, "g": nc.gpsimd, "s": nc.sync,
                    "v": nc.vector}

            def emit_taps(s3, folded, NP, r0, si, pt=None, row_off=0):
                # psum col block j holds output rows [4j, 4j+4) (row-major).
                # Per (pair, tap): 4 quadrant matmuls on blocks {p, p+NP}:
                # img i -> half i on block p, half 1-i on block p+NP, so ot
                # halves end up [img rows 0..R/2 | img rows R/2..R] contiguous.
                if pt is None:
                    pt = ppool.tile([128, 1024 * NP], F32, tag="ps",
                                    name=f"ps{si}")
                ntap = 15 if folded else 9
                tap0 = 9 if folded else 0
                nkx = 5 if folded else 3
                cofs = 0 if folded else 1
                for p in range(NP):
                    for t in range(ntap):
                        ky, kx = t // nkx, t % nkx
                        tw = tap0 + t
                        for (i, j, pbase) in ((0, p, 0), (1, p, 64),
                                              (0, p + NP, 64), (1, p + NP, 0)):
                            rhs = s3[64 * i:64 * i + 64,
                                     row_off + 4 * j + ky:
                                     row_off + 4 * j + ky + 4,
                                     kx + cofs:kx + cofs + 128]
                            nc.tensor.matmul(
                                pt[pbase:pbase + 64, 512 * j:512 * j + 512],
                                wt_sb[64 * i:64 * i + 64, tw * C:(tw + 1) * C],
                                rhs,
                                start=(t == 0), stop=(t == ntap - 1),
                                skip_group_check=True,
                            )
                return pt

            def emit_evac_store(pt, R, NP, r0, split=False):
                ncol = R * W
                hc = 512 * NP   # half of the strip's rows
                ot = opool.tile([128, ncol], ODT, tag="ot", name=f"ot{r0}")
                if NP == 2:
                    dv, dvpc, g = out8, out8pc, r0 // 8
                else:
                    dv, dvpc, g = out4, out4pc, r0 // 4
                stores = [
                    (dvpc[:, g + 0, :], ot[:, 0:hc]),
                    (dv[0, :, g + 1, :], ot[64:128, hc:2 * hc]),
                    (dv[1, :, g + 1, :], ot[0:64, hc:2 * hc]),
                ]
                if split:
                    # tail strips: halves evacuated on ScalarE/DVE in parallel,
                    # stores chase each half; all on HWDGE queues so no
                    # GpSimd SWDGE drain delays the kernel end
                    nc.scalar.copy(out=ot[:, 0:hc], in_=pt[:, 0:hc])
                    nc.sync.dma_start(out=stores[0][0], in_=stores[0][1])
                    nc.vector.tensor_copy(out=ot[:, hc:ncol],
                                          in_=pt[:, hc:ncol])
                    nc.sync.dma_start(out=stores[1][0], in_=stores[1][1])
                    nc.scalar.dma_start(out=stores[2][0], in_=stores[2][1])
                else:
                    nc.scalar.copy(out=ot[:], in_=pt[:, 0:ncol])
                    for (dst, srcp), qc in zip(stores, STORE_Q):
                        qmap[qc].dma_start(out=dst, in_=srcp)

            pend_h2 = None    # (sv3, th3, s3, NP, r0, si) F strip awaiting H2+taps
            evac_fifo = []    # [(pt, R, NP, r0)]
            prev_sv = None    # (tile3, S, is_T): sv-valued buffer of prev strip
            prev_xb = None    # (xb3, nxr): previous strip's cast x tile

            has_indep = any(ind for _, _, ind in PLAN)

            # ---- strip geometry (plan order) ----
            geom = []
            r0n = 0
            for si, (R, folded, indep) in enumerate(PLAN):
                if indep:
                    r0 = H - R
                else:
                    r0 = r0n
                    r0n += R
                if indep:
                    x_lo = r0 - 2
                elif si == 0:
                    x_lo = r0
                else:
                    # ZOV=1: zero-overlap loads (min DMA, +2 small DVE ops);
                    # ZOV=0: 2-row halo re-load (min DVE ops, +1 MB DMA)
                    x_lo = r0 + 2 if ZOV else r0
                x_hi = min(H, r0 + R + 2)
                geom.append((r0, x_lo, x_hi))

            # ---- loads: SWDGE cast-DMA (f32 DRAM -> bf16 SBUF), all
            # pre-emitted so descriptor generation runs once up front and
            # every strip's data streams in as early as HBM bandwidth allows
            pre_xb = [None] * len(PLAN)
            if DMACAST:
                for si in range(len(PLAN)):
                    r0, x_lo, x_hi = geom[si]
                    nxr = x_hi - x_lo
                    xb = bpool.tile([128, nxr * W], BF16, tag="xb")
                    # all loads on the single Pool SWDGE ring: FIFO per ring,
                    # so strip 0 streams first at full bandwidth
                    nc.gpsimd.dma_start(out=xb[:], in_=x_pc[:, x_lo:x_hi, :])
                    pre_xb[si] = xb

            for si, (R, folded, indep) in enumerate(PLAN):
                first = si == 0
                r0, x_lo, x_hi = geom[si]
                # geometric bottom strip: handles row-127 reflect + zero row
                last = indep or (not has_indep and si == len(PLAN) - 1)
                r1 = r0 + R
                S = R + 2                     # s rows: positions [r0-1, r1+1)
                NP = R * W // 1024

                nxr = x_hi - x_lo
                if DMACAST:
                    xb = pre_xb[si]
                else:
                    xt = xpool.tile([128, nxr * W], F32, tag="xt")
                    xb = bpool.tile([128, nxr * W], BF16, tag="xb")
                    if first:
                        # split load + DVE cast (no ACT_TABLE_LOAD dependency)
                        # so the first strip's chain starts as early as can be
                        q0a = nc.scalar if L0Q == "a" else nc.sync
                        q0b = nc.scalar if L0Q in "ab" else nc.sync
                        mid = nxr // 2
                        q0a.dma_start(out=xt[:, 0:mid * W],
                                      in_=x_pc[:, x_lo:x_lo + mid, :])
                        q0b.dma_start(out=xt[:, mid * W:],
                                      in_=x_pc[:, x_lo + mid:x_hi, :])
                        nc.vector.tensor_copy(out=xb[:, 0:mid * W],
                                              in_=xt[:, 0:mid * W])
                        nc.vector.tensor_copy(out=xb[:, mid * W:],
                                              in_=xt[:, mid * W:])
                    else:
                        nc.sync.dma_start(out=xt[:], in_=x_pc[:, x_lo:x_hi, :])
                        nc.scalar.copy(out=xb[:], in_=xt[:])
                xb3 = xb.rearrange("p (r w) -> p r w", w=W)

                # ---- V-box: sv[j] = x[j-1]+x[j]+x[j+1], reflect at 0/127 ----
                jlo = 0 if first else (r0 - 1 if indep else r0 + 1)
                jhi = H if last else r1 + 1
                nu = jhi - jlo
                jm_lo = max(jlo, x_lo + 1)
                jm_hi = min(jhi, x_hi - 1)
                ut = upool.tile([128, nu * W], BF16, tag="ut")
                u3 = ut.rearrange("p (r w) -> p r w", w=W)
                nc.vector.tensor_add(
                    out=u3[:, jm_lo - jlo:jm_hi - jlo, :],
                    in0=xb3[:, jm_lo - 1 - x_lo:jm_hi - 1 - x_lo, :],
                    in1=xb3[:, jm_lo + 1 - x_lo:jm_hi + 1 - x_lo, :])
                xtile = jm_lo > jlo and not first and not indep
                if xtile:
                    # boundary rows j in [jlo, jm_lo): x[j-1] from prev tile
                    pxb3, pnxr = prev_xb
                    nb = jm_lo - jlo   # == 2
                    nc.vector.tensor_add(
                        out=u3[:, 0:nb, :],
                        in0=pxb3[:, pnxr - nb:pnxr, :],
                        in1=xb3[:, jlo + 1 - x_lo:jm_lo + 1 - x_lo, :])
                if first:
                    nc.vector.tensor_scalar_mul(
                        out=u3[:, 0:1, :], in0=xb3[:, 1:2, :], scalar1=2.0)
                if last:
                    nc.vector.tensor_scalar_mul(
                        out=u3[:, nu - 1:nu, :],
                        in0=xb3[:, 126 - x_lo:127 - x_lo, :], scalar1=2.0)
                if not indep:
                    prev_xb = (xb3, nxr)

                st = spool.tile([128, S * WP], BF16, tag="st")
                s3 = st.rearrange("p (r c) -> p r c", c=WP)
                brow = jlo - (r0 - 1)
                if folded:
                    sv3 = None
                    v2out = lambda a, b: s3[:, brow + a:brow + b, 2:130]
                else:
                    svt = vpool.tile([128, S * W], BF16, tag="svt")
                    sv3 = svt.rearrange("p (r w) -> p r w", w=W)
                    v2out = lambda a, b: sv3[:, brow + a:brow + b, :]
                if xtile:
                    # sv[jlo]: center tap x[jlo] is prev tile's last row
                    nc.vector.tensor_add(
                        out=v2out(0, 1), in0=u3[:, 0:1, :],
                        in1=pxb3[:, pnxr - 1:pnxr, :])
                    nc.vector.tensor_add(
                        out=v2out(1, nu), in0=u3[:, 1:nu, :],
                        in1=xb3[:, jlo + 1 - x_lo:jhi - x_lo, :])
                else:
                    nc.vector.tensor_add(
                        out=v2out(0, nu), in0=u3[:, :, :],
                        in1=xb3[:, jlo - x_lo:jhi - x_lo, :])

                # ---- head rows [r0-1, r0+1): copy prev strip's last 2 sv rows
                if not first and not indep:
                    psv3, pS, p_folded = prev_sv
                    src = (psv3[:, pS - 2:pS, 2:130] if p_folded
                           else psv3[:, pS - 2:pS, :])
                    dst = (s3[:, 0:2, 2:130] if folded else sv3[:, 0:2, :])
                    if HEAD_SC:
                        nc.scalar.copy(out=dst, in_=src)
                    else:
                        nc.vector.tensor_copy(out=dst, in_=src)
                if not indep:
                    prev_sv = ((s3, S, True) if folded else (sv3, S, False))

                # ---- conv zero-pad rows at image top/bottom (before the
                # side-col fixups, which read all rows) ----
                if first:
                    nc.vector.memset(s3[:, 0:1, :], 0.0)
                if last:
                    nc.vector.memset(s3[:, S - 1:S, :], 0.0)

                # valid sv rows for th/H2: row 0 is the conv zero row on the
                # first strip, row S-1 on the geometric bottom strip
                h2lo = brow if first else 0
                h2hi = S - 1 if last else S
                if not folded:
                    # ---- H-box pair-add: th[w] = sv[w-1]+sv[w+1] ----
                    tht = tpool.tile([128, S * W], BF16, tag="tht")
                    th3 = tht.rearrange("p (r w) -> p r w", w=W)
                    hp = max(h2lo, min(h2hi, h2lo + H1_POOL_ROWS))
                    if hp > h2lo:
                        nc.gpsimd.tensor_add(out=th3[:, h2lo:hp, 1:127],
                                             in0=sv3[:, h2lo:hp, 0:126],
                                             in1=sv3[:, h2lo:hp, 2:128])
                    if hp < h2hi:
                        nc.vector.tensor_add(out=th3[:, hp:h2hi, 1:127],
                                             in0=sv3[:, hp:h2hi, 0:126],
                                             in1=sv3[:, hp:h2hi, 2:128])
                    if EDGE_SC:
                        nc.scalar.mul(th3[:, h2lo:h2hi, 0:1],
                                      sv3[:, h2lo:h2hi, 1:2], 2.0)
                        nc.scalar.mul(th3[:, h2lo:h2hi, 127:128],
                                      sv3[:, h2lo:h2hi, 126:127], 2.0)
                    else:
                        nc.vector.tensor_scalar_mul(
                            out=th3[:, h2lo:h2hi, 0:1],
                            in0=sv3[:, h2lo:h2hi, 1:2], scalar1=2.0)
                        nc.vector.tensor_scalar_mul(
                            out=th3[:, h2lo:h2hi, 127:128],
                            in0=sv3[:, h2lo:h2hi, 126:127], scalar1=2.0)
                    nc.vector.memset(s3[:, :, 0:2], 0.0)
                    nc.vector.memset(s3[:, :, 130:132], 0.0)
                else:
                    # ---- folded side cols: col c reads sv[c-2] ----
                    nc.vector.tensor_copy(out=s3[:, :, 1:2], in_=s3[:, :, 3:4])
                    nc.vector.tensor_copy(out=s3[:, :, 130:131],
                                          in_=s3[:, :, 128:129])
                    nc.vector.scalar_tensor_tensor(
                        out=s3[:, :, 0:1], in0=s3[:, :, 2:3], scalar=-1.0,
                        in1=s3[:, :, 3:4], op0=mybir.AluOpType.mult,
                        op1=mybir.AluOpType.subtract)
                    nc.vector.scalar_tensor_tensor(
                        out=s3[:, :, 131:132], in0=s3[:, :, 129:130],
                        scalar=-1.0, in1=s3[:, :, 128:129],
                        op0=mybir.AluOpType.mult,
                        op1=mybir.AluOpType.subtract)

                if folded:
                    for h, off in enumerate(range(0, R, CHUNK)):
                        rows = min(CHUNK, R - off)
                        cnp = rows * W // 1024
                        pt = emit_taps(s3, True, cnp, r0 + off, 10 * si + h,
                                       pt=warm_pt if (first and h == 0)
                                       else None, row_off=off)
                        evac_fifo.append((pt, rows, cnp, r0 + off))
                        while len(evac_fifo) > 1:
                            emit_evac_store(*evac_fifo.pop(0))

                def emit_h2_taps(dsv3, dth3, ds3, dR, dr0, dsi, dlo, dhi):
                    nc.vector.tensor_add(out=ds3[:, dlo:dhi, 2:130],
                                         in0=dth3[:, dlo:dhi, 0:128],
                                         in1=dsv3[:, dlo:dhi, 0:128])
                    # taps/evac/stores in CHUNK-row blocks (psum slots)
                    for h, off in enumerate(range(0, dR, CHUNK)):
                        rows = min(CHUNK, dR - off)
                        cnp = rows * W // 1024
                        pt = emit_taps(ds3, False, cnp, dr0 + off,
                                       10 * dsi + h, row_off=off)
                        evac_fifo.append((pt, rows, cnp, dr0 + off))
                        while len(evac_fifo) > 1:
                            emit_evac_store(*evac_fifo.pop(0))

                # ---- deferred H2 + taps of the previous F strip (emitted
                # after a folded strip's taps so those hide under the H2) ----
                if pend_h2 is not None:
                    emit_h2_taps(*pend_h2)
                    pend_h2 = None

                if folded:
                    pass
                elif H1_POOL_ROWS == 0 and not (si + 1 < len(PLAN)
                                                and PLAN[si + 1][1]):
                    # H1 all on DVE: no cross-engine stall risk, emit inline
                    emit_h2_taps(sv3, th3, s3, R, r0, si, h2lo, h2hi)
                else:
                    # last F strip (or H1_POOL_ROWS mode): defer H2+taps
                    pend_h2 = (sv3, th3, s3, R, r0, si, h2lo, h2hi)

                while len(evac_fifo) > 1:
                    emit_evac_store(*evac_fifo.pop(0))

            if pend_h2 is not None:
                emit_h2_taps(*pend_h2)
            while evac_fifo:
                emit_evac_store(*evac_fifo.pop(0), split=True)

    nc.compile()
    return nc


_CACHE = {}


def _get_nc():
    if "nc" not in _CACHE:
        _CACHE["nc"] = build()
    return _CACHE["nc"]


def kernel(x: np.ndarray, W: np.ndarray, trace: bool = False):
    x = np.asarray(x, dtype=np.float32)
    wt = _host_weights(np.asarray(W, dtype=np.float32))
    nc = _get_nc()
    core_ids = list(range(N_CORES))
    in_maps = [
        {"x": np.ascontiguousarray(x[BPC * i:BPC * (i + 1)]), "wt": wt}
        for i in core_ids
    ]
    res = run_bass_kernel_spmd(nc, in_maps, core_ids, trace=trace)
    out = np.concatenate(
        [res.results[i]["out"].astype(np.float32) for i in core_ids], axis=0)
    if trace:
        kernel.last_exec_time_ns = res.exec_time_ns
        kernel.last_res = res
    return out


kernel.last_exec_time_ns = None



# revision 24
# speedup vs baseline: 1.0924x; 1.0047x over previous
"""Trainium2 kernel for Conv2d_cd (central-difference conv, 3x3, theta=0.7).

Reference math:
    s = sum of 9 shifted views of reflect-padded x  (= 3x3 box filter, reflect pad)
    out = conv3x3_zeropad(s, W) - theta * conv1x1(s, W.sum((2,3)))
        = conv3x3_zeropad(s, W')     with W'[:,:,1,1] -= theta * W.sum((2,3))

Strategy (per NeuronCore, 8 cores data-parallel over batch 16 -> 2 images/core):
  - images stacked on SBUF partition halves: partitions 0:64 = img0 ch, 64:128 = img1 ch
  - H strips, all 9-tap (plan F8,J8,F16*6,F8,F8; J = independent unfolded
    bottom strip computed early so the drain tail is short).  Per strip:
    HWDGE f32 load, cast x->bf16 on ScalarE (DVE for strip 0), V-box +
    H-box + center-add as bf16 2x adds on DVE (edge/head fixups kept on
    DVE: putting them on ACT lets the Tile scheduler order them ahead of
    PSUM-freeing evacuations).  Strips share their 2 boundary s rows via
    a small copy instead of recomputation.
  - 14 dep-free warmup matmuls run during the load phase so the PE HAM
    clock is at 8/8 before the first real tap (~3us saved vs cold start).
  - taps as K=64/M=64 matmuls packed 4-concurrent in PE quadrants, emitted
    in 8-row chunks into [128,1024] PSUM tiles (2 banks) rotating over 4
    pool slots, so the PE can run several chunks ahead of evacuation.
    Evacuation: ScalarE copy to fp16 (output stored as fp16, widened to
    fp32 on host; rel-err budget 2e-2); stores on sync/gpsimd rings.
"""

import os

import numpy as np
import ml_dtypes

import concourse.bass as bass
import concourse.bacc as bacc
import concourse.mybir as mybir
from concourse.tile import TileContext
from concourse.bass_utils import run_bass_kernel_spmd

THETA = 0.7
N_CORES = 8
B, C, H, W = 16, 64, 128, 128
BPC = B // N_CORES          # images per core = 2
WP = W + 4                  # padded width of s tiles (132)
F32 = mybir.dt.float32
BF16 = mybir.dt.bfloat16
F16 = mybir.dt.float16


def _parse_plan():
    # kinds: T = folded (15-tap), F = unfolded (9-tap + H on DVE),
    # I = independent folded bottom strip (rows [128-R, 128), computes its
    # own boundary rows; lets the bottom taps run early instead of last)
    txt = os.environ.get("K_PLAN", "F8,J8,F16,F16,F16,F16,F16,F16,F8,F8")
    plan = []
    for item in txt.split(","):
        item = item.strip()
        plan.append((int(item[1:]), item[0] in "TI", item[0] in "IJ"))
    assert sum(r for r, _, _ in plan) == H, plan
    return plan


PLAN = _parse_plan()
OUT16 = os.environ.get("K_OUT16", "1") == "1"   # store output as fp16
H1_POOL_ROWS = int(os.environ.get("K_H1_POOL_ROWS", "0"))  # of S=18 on GpSimd
STORE_Q = os.environ.get("K_STORE_Q", "ssg")  # a=scalar g=gpsimd s=sync v=vector
DMACAST = os.environ.get("K_DMACAST", "0") == "1"  # f32->bf16 cast in SWDGE DMA
XBUFS = int(os.environ.get("K_XBUFS", str(len(PLAN) if DMACAST else 4)))
SBUFS = int(os.environ.get("K_SBUFS", "4"))
OBUFS = int(os.environ.get("K_OBUFS", "3"))
PBUFS = int(os.environ.get("K_PBUFS", "4"))    # psum rotation slots (2 banks ea)
CHUNK = int(os.environ.get("K_CHUNK", "8"))    # tap/evac chunk rows
ODT = F16 if OUT16 else F32
EDGE_SC = os.environ.get("K_EDGE_SC", "1" if DMACAST else "0") == "1"
ZOV = os.environ.get("K_ZOV", "0") == "1"
HEAD_SC = os.environ.get("K_HEAD_SC", "1" if DMACAST else "0") == "1"
L0Q = os.environ.get("K_L0Q", "s")  # first-strip load ring: s=sync a=scalar


def _host_weights(Wnp: np.ndarray):
    """W' and packed tap weights: wt[p, t*C + co], taps 0..8 = w9 (ky*3+kx),
    taps 9..23 = w15 (ky*5+tx); both partition halves identical."""
    Wp = Wnp.astype(np.float64).copy()
    Wp[:, :, 1, 1] -= THETA * Wnp.astype(np.float64).sum(axis=(2, 3))
    w9 = Wp.transpose(1, 2, 3, 0).reshape(C, 9, C)
    w15 = np.zeros((C, 3, 5, C), np.float64)
    for ky in range(3):
        for tx in range(5):
            for kx in range(max(0, tx - 2), min(2, tx) + 1):
                w15[:, ky, tx, :] += Wp[:, :, ky, kx].T  # [ci, co]
    wt = np.concatenate([w9.reshape(C, 9 * C), w15.reshape(C, 15 * C)], axis=1)
    wt = np.concatenate([wt, wt], axis=0)
    return np.ascontiguousarray(wt.astype(ml_dtypes.bfloat16))


def build():
    nc = bacc.Bacc("TRN2", target_bir_lowering=False, debug=False,
                   num_devices=N_CORES)
    x_d = nc.declare_dram_parameter("x", [BPC, C, H, W], F32, isOutput=False)
    wt_d = nc.declare_dram_parameter("wt", [128, 24 * C], BF16, isOutput=False)
    out_d = nc.declare_dram_parameter("out", [BPC, C, H, W], ODT, isOutput=True)

    x_pc = x_d.rearrange("i c h w -> (i c) h w")
    # store views: 8 (or 4) contiguous rows per (img, ch) descriptor
    out8 = out_d.rearrange("i c (g8 eight) w -> i c g8 (eight w)", eight=8)
    out4 = out_d.rearrange("i c (g4 four) w -> i c g4 (four w)", four=4)
    out8pc = out_d.rearrange("i c (g8 eight) w -> (i c) g8 (eight w)", eight=8)
    out4pc = out_d.rearrange("i c (g4 four) w -> (i c) g4 (four w)", four=4)

    pam = os.environ.get("K_POOL_MODE", "queue")
    with TileContext(nc, pool_alloc_mode=pam) as tc:
        with (
            tc.tile_pool(name="wpool", bufs=1) as wpool,
            tc.tile_pool(name="warmpool", bufs=1) as warmpool,
            tc.tile_pool(name="xpool", bufs=XBUFS) as xpool,
            tc.tile_pool(name="bpool", bufs=XBUFS) as bpool,
            tc.tile_pool(name="upool", bufs=2) as upool,
            tc.tile_pool(name="vpool", bufs=2) as vpool,
            tc.tile_pool(name="tpool", bufs=2) as tpool,
            tc.tile_pool(name="spool", bufs=SBUFS) as spool,
            tc.tile_pool(name="opool", bufs=OBUFS) as opool,
            tc.tile_pool(name="psum", bufs=PBUFS, space="PSUM") as ppool,
        ):
            wt_sb = wpool.tile([128, 24 * C], BF16)
            (nc.sync if L0Q in "ab" else nc.scalar).dma_start(
                out=wt_sb[:], in_=wt_d[:])

            # PE p-state warmup: dep-free matmuls keep the tensor engine
            # busy through the load phase so the first real taps run at
            # full clock (ramp needs ~3us of sustained execution)
            NWARM = int(os.environ.get("K_WARM", "14"))
            warm_pt = None
            if NWARM:
                wrm = warmpool.tile([128, 512], BF16)
                nc.vector.memset(wrm[:], 0.0)
                NP0 = PLAN[0][0] * W // 1024
                warm_pt = ppool.tile([128, 1024 * NP0], F32, tag="ps",
                                     name="warm_ps")
                for _ in range(NWARM):
                    nc.tensor.matmul(warm_pt[0:64, 0:512], wrm[0:64, 0:64],
                                     wrm[0:64, :], start=True, stop=True,
                                     skip_group_check=True)

            qmap = {"a": nc.scalar, "g": nc.gpsimd, "s": nc.sync,
                    "v": nc.vector}

            def emit_taps(s3, folded, NP, r0, si, pt=None, row_off=0):
                # psum col block j holds output rows [4j, 4j+4) (row-major).
                # Per (pair, tap): 4 quadrant matmuls on blocks {p, p+NP}:
                # img i -> half i on block p, half 1-i on block p+NP, so ot
                # halves end up [img rows 0..R/2 | img rows R/2..R] contiguous.
                if pt is None:
                    pt = ppool.tile([128, 1024 * NP], F32, tag="ps",
                                    name=f"ps{si}")
                ntap = 15 if folded else 9
                tap0 = 9 if folded else 0
                nkx = 5 if folded else 3
                cofs = 0 if folded else 1
                for p in range(NP):
                    for t in range(ntap):
                        ky, kx = t // nkx, t % nkx
                        tw = tap0 + t
                        for (i, j, pbase) in ((0, p, 0), (1, p, 64),
                                              (0, p + NP, 64), (1, p + NP, 0)):
                            rhs = s3[64 * i:64 * i + 64,
                                     row_off + 4 * j + ky:
                                     row_off + 4 * j + ky + 4,
                                     kx + cofs:kx + cofs + 128]
                            nc.tensor.matmul(
                                pt[pbase:pbase + 64, 512 * j:512 * j + 512],
                                wt_sb[64 * i:64 * i + 64, tw * C:(tw + 1) * C],
                                rhs,
                                start=(t == 0), stop=(t == ntap - 1),
                                skip_group_check=True,
                            )
                return pt

            def emit_evac_store(pt, R, NP, r0, split=False):
                ncol = R * W
                hc = 512 * NP   # half of the strip's rows
                ot = opool.tile([128, ncol], ODT, tag="ot", name=f"ot{r0}")
                if NP == 2:
                    dv, dvpc, g = out8, out8pc, r0 // 8
                else:
                    dv, dvpc, g = out4, out4pc, r0 // 4
                stores = [
                    (dvpc[:, g + 0, :], ot[:, 0:hc]),
                    (dv[0, :, g + 1, :], ot[64:128, hc:2 * hc]),
                    (dv[1, :, g + 1, :], ot[0:64, hc:2 * hc]),
                ]
                if split:
                    # tail strips: halves evacuated on ScalarE/DVE in parallel,
                    # stores chase each half; all on HWDGE queues so no
                    # GpSimd SWDGE drain delays the kernel end
                    nc.scalar.copy(out=ot[:, 0:hc], in_=pt[:, 0:hc])
                    nc.sync.dma_start(out=stores[0][0], in_=stores[0][1])
                    nc.vector.tensor_copy(out=ot[:, hc:ncol],
                                          in_=pt[:, hc:ncol])
                    nc.sync.dma_start(out=stores[1][0], in_=stores[1][1])
                    nc.scalar.dma_start(out=stores[2][0], in_=stores[2][1])
                else:
                    nc.scalar.copy(out=ot[:], in_=pt[:, 0:ncol])
                    for (dst, srcp), qc in zip(stores, STORE_Q):
                        qmap[qc].dma_start(out=dst, in_=srcp)

            pend_h2 = None    # (sv3, th3, s3, NP, r0, si) F strip awaiting H2+taps
            evac_fifo = []    # [(pt, R, NP, r0)]
            prev_sv = None    # (tile3, S, is_T): sv-valued buffer of prev strip
            prev_xb = None    # (xb3, nxr): previous strip's cast x tile

            has_indep = any(ind for _, _, ind in PLAN)

            # ---- strip geometry (plan order) ----
            geom = []
            r0n = 0
            for si, (R, folded, indep) in enumerate(PLAN):
                if indep:
                    r0 = H - R
                else:
                    r0 = r0n
                    r0n += R
                if indep:
                    x_lo = r0 - 2
                elif si == 0:
                    x_lo = r0
                else:
                    # ZOV=1: zero-overlap loads (min DMA, +2 small DVE ops);
                    # ZOV=0: 2-row halo re-load (min DVE ops, +1 MB DMA)
                    x_lo = r0 + 2 if ZOV else r0
                x_hi = min(H, r0 + R + 2)
                geom.append((r0, x_lo, x_hi))

            # ---- loads: SWDGE cast-DMA (f32 DRAM -> bf16 SBUF), all
            # pre-emitted so descriptor generation runs once up front and
            # every strip's data streams in as early as HBM bandwidth allows
            pre_xb = [None] * len(PLAN)
            if DMACAST:
                for si in range(len(PLAN)):
                    r0, x_lo, x_hi = geom[si]
                    nxr = x_hi - x_lo
                    xb = bpool.tile([128, nxr * W], BF16, tag="xb")
                    # all loads on the single Pool SWDGE ring: FIFO per ring,
                    # so strip 0 streams first at full bandwidth
                    nc.gpsimd.dma_start(out=xb[:], in_=x_pc[:, x_lo:x_hi, :])
                    pre_xb[si] = xb

            for si, (R, folded, indep) in enumerate(PLAN):
                first = si == 0
                r0, x_lo, x_hi = geom[si]
                # geometric bottom strip: handles row-127 reflect + zero row
                last = indep or (not has_indep and si == len(PLAN) - 1)
                r1 = r0 + R
                S = R + 2                     # s rows: positions [r0-1, r1+1)
                NP = R * W // 1024

                nxr = x_hi - x_lo
                if DMACAST:
                    xb = pre_xb[si]
                else:
                    xt = xpool.tile([128, nxr * W], F32, tag="xt")
                    xb = bpool.tile([128, nxr * W], BF16, tag="xb")
                    if first:
                        # split load + DVE cast (no ACT_TABLE_LOAD dependency)
                        # so the first strip's chain starts as early as can be
                        q0a = nc.scalar if L0Q == "a" else nc.sync
                        q0b = nc.scalar if L0Q in "ab" else nc.sync
                        mid = nxr // 2
                        q0a.dma_start(out=xt[:, 0:mid * W],
                                      in_=x_pc[:, x_lo:x_lo + mid, :])
                        q0b.dma_start(out=xt[:, mid * W:],
                                      in_=x_pc[:, x_lo + mid:x_hi, :])
                        nc.vector.tensor_copy(out=xb[:, 0:mid * W],
                                              in_=xt[:, 0:mid * W])
                        nc.vector.tensor_copy(out=xb[:, mid * W:],
                                              in_=xt[:, mid * W:])
                    else:
                        nc.sync.dma_start(out=xt[:], in_=x_pc[:, x_lo:x_hi, :])
                        nc.scalar.copy(out=xb[:], in_=xt[:])
                xb3 = xb.rearrange("p (r w) -> p r w", w=W)

                # ---- V-box: sv[j] = x[j-1]+x[j]+x[j+1], reflect at 0/127 ----
                jlo = 0 if first else (r0 - 1 if indep else r0 + 1)
                jhi = H if last else r1 + 1
                nu = jhi - jlo
                jm_lo = max(jlo, x_lo + 1)
                jm_hi = min(jhi, x_hi - 1)
                ut = upool.tile([128, nu * W], BF16, tag="ut")
                u3 = ut.rearrange("p (r w) -> p r w", w=W)
                nc.vector.tensor_add(
                    out=u3[:, jm_lo - jlo:jm_hi - jlo, :],
                    in0=xb3[:, jm_lo - 1 - x_lo:jm_hi - 1 - x_lo, :],
                    in1=xb3[:, jm_lo + 1 - x_lo:jm_hi + 1 - x_lo, :])
                xtile = jm_lo > jlo and not first and not indep
                if xtile:
                    # boundary rows j in [jlo, jm_lo): x[j-1] from prev tile
                    pxb3, pnxr = prev_xb
                    nb = jm_lo - jlo   # == 2
                    nc.vector.tensor_add(
                        out=u3[:, 0:nb, :],
                        in0=pxb3[:, pnxr - nb:pnxr, :],
                        in1=xb3[:, jlo + 1 - x_lo:jm_lo + 1 - x_lo, :])
                if first:
                    nc.vector.tensor_scalar_mul(
                        out=u3[:, 0:1, :], in0=xb3[:, 1:2, :], scalar1=2.0)
                if last:
                    nc.vector.tensor_scalar_mul(
                        out=u3[:, nu - 1:nu, :],
                        in0=xb3[:, 126 - x_lo:127 - x_lo, :], scalar1=2.0)
                if not indep:
                    prev_xb = (xb3, nxr)

                st = spool.tile([128, S * WP], BF16, tag="st")
                s3 = st.rearrange("p (r c) -> p r c", c=WP)
                brow = jlo - (r0 - 1)
                if folded:
                    sv3 = None
                    v2out = lambda a, b: s3[:, brow + a:brow + b, 2:130]
                else:
                    svt = vpool.tile([128, S * W], BF16, tag="svt")
                    sv3 = svt.rearrange("p (r w) -> p r w", w=W)
                    v2out = lambda a, b: sv3[:, brow + a:brow + b, :]
                if xtile:
                    # sv[jlo]: center tap x[jlo] is prev tile's last row
                    nc.vector.tensor_add(
                        out=v2out(0, 1), in0=u3[:, 0:1, :],
                        in1=pxb3[:, pnxr - 1:pnxr, :])
                    nc.vector.tensor_add(
                        out=v2out(1, nu), in0=u3[:, 1:nu, :],
                        in1=xb3[:, jlo + 1 - x_lo:jhi - x_lo, :])
                else:
                    nc.vector.tensor_add(
                        out=v2out(0, nu), in0=u3[:, :, :],
                        in1=xb3[:, jlo - x_lo:jhi - x_lo, :])

                # ---- head rows [r0-1, r0+1): copy prev strip's last 2 sv rows
                if not first and not indep:
                    psv3, pS, p_folded = prev_sv
                    src = (psv3[:, pS - 2:pS, 2:130] if p_folded
                           else psv3[:, pS - 2:pS, :])
                    dst = (s3[:, 0:2, 2:130] if folded else sv3[:, 0:2, :])
                    if HEAD_SC:
                        nc.scalar.copy(out=dst, in_=src)
                    else:
                        nc.vector.tensor_copy(out=dst, in_=src)
                if not indep:
                    prev_sv = ((s3, S, True) if folded else (sv3, S, False))

                # ---- conv zero-pad rows at image top/bottom (before the
                # side-col fixups, which read all rows) ----
                if first:
                    nc.vector.memset(s3[:, 0:1, :], 0.0)
                if last:
                    nc.vector.memset(s3[:, S - 1:S, :], 0.0)

                # valid sv rows for th/H2: row 0 is the conv zero row on the
                # first strip, row S-1 on the geometric bottom strip
                h2lo = brow if first else 0
                h2hi = S - 1 if last else S
                if not folded:
                    # ---- H-box pair-add: th[w] = sv[w-1]+sv[w+1] ----
                    tht = tpool.tile([128, S * W], BF16, tag="tht")
                    th3 = tht.rearrange("p (r w) -> p r w", w=W)
                    hp = max(h2lo, min(h2hi, h2lo + H1_POOL_ROWS))
                    if hp > h2lo:
                        nc.gpsimd.tensor_add(out=th3[:, h2lo:hp, 1:127],
                                             in0=sv3[:, h2lo:hp, 0:126],
                                             in1=sv3[:, h2lo:hp, 2:128])
                    if hp < h2hi:
                        nc.vector.tensor_add(out=th3[:, hp:h2hi, 1:127],
                                             in0=sv3[:, hp:h2hi, 0:126],
                                             in1=sv3[:, hp:h2hi, 2:128])
                    if EDGE_SC:
                        nc.scalar.mul(th3[:, h2lo:h2hi, 0:1],
                                      sv3[:, h2lo:h2hi, 1:2], 2.0)
                        nc.scalar.mul(th3[:, h2lo:h2hi, 127:128],
                                      sv3[:, h2lo:h2hi, 126:127], 2.0)
                    else:
                        nc.vector.tensor_scalar_mul(
                            out=th3[:, h2lo:h2hi, 0:1],
                            in0=sv3[:, h2lo:h2hi, 1:2], scalar1=2.0)
                        nc.vector.tensor_scalar_mul(
                            out=th3[:, h2lo:h2hi, 127:128],
                            in0=sv3[:, h2lo:h2hi, 126:127], scalar1=2.0)
                    nc.vector.memset(s3[:, :, 0:2], 0.0)
                    nc.vector.memset(s3[:, :, 130:132], 0.0)
                else:
                    # ---- folded side cols: col c reads sv[c-2] ----
                    nc.vector.tensor_copy(out=s3[:, :, 1:2], in_=s3[:, :, 3:4])
                    nc.vector.tensor_copy(out=s3[:, :, 130:131],
                                          in_=s3[:, :, 128:129])
                    nc.vector.scalar_tensor_tensor(
                        out=s3[:, :, 0:1], in0=s3[:, :, 2:3], scalar=-1.0,
                        in1=s3[:, :, 3:4], op0=mybir.AluOpType.mult,
                        op1=mybir.AluOpType.subtract)
                    nc.vector.scalar_tensor_tensor(
                        out=s3[:, :, 131:132], in0=s3[:, :, 129:130],
                        scalar=-1.0, in1=s3[:, :, 128:129],
                        op0=mybir.AluOpType.mult,
                        op1=mybir.AluOpType.subtract)

                if folded:
                    for h, off in enumerate(range(0, R, CHUNK)):
                        rows = min(CHUNK, R - off)
                        cnp = rows * W // 1024
                        pt = emit_taps(s3, True, cnp, r0 + off, 10 * si + h,
                                       pt=warm_pt if (first and h == 0)
                                       else None, row_off=off)
                        evac_fifo.append((pt, rows, cnp, r0 + off))
                        while len(evac_fifo) > 1:
                            emit_evac_store(*evac_fifo.pop(0))

                def emit_h2_taps(dsv3, dth3, ds3, dR, dr0, dsi, dlo, dhi):
                    nc.vector.tensor_add(out=ds3[:, dlo:dhi, 2:130],
                                         in0=dth3[:, dlo:dhi, 0:128],
                                         in1=dsv3[:, dlo:dhi, 0:128])
                    # taps/evac/stores in CHUNK-row blocks (psum slots)
                    for h, off in enumerate(range(0, dR, CHUNK)):
                        rows = min(CHUNK, dR - off)
                        cnp = rows * W // 1024
                        pt = emit_taps(ds3, False, cnp, dr0 + off,
                                       10 * dsi + h, row_off=off)
                        evac_fifo.append((pt, rows, cnp, dr0 + off))
                        while len(evac_fifo) > 1:
                            emit_evac_store(*evac_fifo.pop(0))

                # ---- deferred H2 + taps of the previous F strip (emitted
                # after a folded strip's taps so those hide under the H2) ----
                if pend_h2 is not None:
                    emit_h2_taps(*pend_h2)
                    pend_h2 = None

                if folded:
                    pass
                elif H1_POOL_ROWS == 0 and not (si + 1 < len(PLAN)
                                                and PLAN[si + 1][1]):
                    # H1 all on DVE: no cross-engine stall risk, emit inline
                    emit_h2_taps(sv3, th3, s3, R, r0, si, h2lo, h2hi)
                else:
                    # last F strip (or H1_POOL_ROWS mode): defer H2+taps
                    pend_h2 = (sv3, th3, s3, R, r0, si, h2lo, h2hi)

                while len(evac_fifo) > 1:
                    emit_evac_store(*evac_fifo.pop(0))

            if pend_h2 is not None:
                emit_h2_taps(*pend_h2)
            while evac_fifo:
                emit_evac_store(*evac_fifo.pop(0), split=True)

    nc.compile()
    return nc


_CACHE = {}


def _get_nc():
    if "nc" not in _CACHE:
        _CACHE["nc"] = build()
    return _CACHE["nc"]


def kernel(x: np.ndarray, W: np.ndarray, trace: bool = False):
    x = np.asarray(x, dtype=np.float32)
    wt = _host_weights(np.asarray(W, dtype=np.float32))
    nc = _get_nc()
    core_ids = list(range(N_CORES))
    in_maps = [
        {"x": np.ascontiguousarray(x[BPC * i:BPC * (i + 1)]), "wt": wt}
        for i in core_ids
    ]
    res = run_bass_kernel_spmd(nc, in_maps, core_ids, trace=trace)
    out = np.concatenate(
        [res.results[i]["out"].astype(np.float32) for i in core_ids], axis=0)
    if trace:
        kernel.last_exec_time_ns = res.exec_time_ns
        kernel.last_res = res
    return out


kernel.last_exec_time_ns = None

